# revision 1
# baseline (speedup 1.0000x reference)
"""Trainium2 Bass kernel: nn_BV_Model (GENConv GNN, softmax aggregation, 4 layers).

Strategy (8 NeuronCores, SPMD):
  - Nodes are partitioned into 8 contiguous blocks (12544/core, padded).
  - Edges are sorted by destination node and bucketed per destination
    node-tile (128 nodes); each core owns the edges whose dst falls in its
    block.  Per-tile edge lists are padded to a multiple of 128 so that
    128-edge chunks never straddle node tiles (the chunk count per tile g is
    shared across cores so one SPMD program fits all cores).
  - Per layer: gather h[src] via indirect DMA with accumulate(+e) directly
    onto the preloaded edge-feature tile, compute the segment softmax
    numerator/denominator with exp (no segment-max: ranges are small enough
    for fp32, verified offline), and reduce edges->nodes with a
    one-hot(dst) matmul accumulated in PSUM.  Node MLP runs on the tensor
    engine in transposed layout.  h is AllGathered across cores per layer.
  - Global mean pool is computed per-core with a one-hot(graph) matmul,
    scattered into a [PG,C] buffer and AllReduced; every core then applies
    the output head redundantly.

Fake-quantization q(x) = clip(rne(x*1024), -32768, 32767)/1024 is computed
exactly with the round-to-nearest-even "magic number" trick (+1.5*2^23).
"""

import os
os.environ.setdefault("MYCRO_LOCAL_CACHE", "1")

import math
import numpy as np

import concourse.bacc as bacc
import concourse.tile as tile
import concourse.bass as bass
from concourse import mybir
from concourse.bass import IndirectOffsetOnAxis
from concourse.bass_utils import run_bass_kernel_spmd

F32 = mybir.dt.float32
I32 = mybir.dt.int32
ACTF = mybir.ActivationFunctionType
AL = mybir.AluOpType

MAGIC = 12582912.0           # 1.5*2^23 : fp32 RNE rounding magic
QS = 1024.0                  # 2^10
QI = 1.0 / 1024.0
QB2 = -12288.0               # -MAGIC * 2^-10
QMAX = 32767.0 / 1024.0
QMIN = -32.0
GEN_EPS = 1e-7
BN_EPS = 1e-5
NCORES = 8

LAST_RESULTS = None          # BassKernelResults of the most recent run (for test.py)


class Cfg:
    def __init__(self, N, E, G, XD=8, ED=4, C=32, L=4, ncores=NCORES,
                 use_collectives=True, use_shared=True, gather_k=None,
                 no_indirect=False, gather_plain=True, n_layers=None):
        self.N, self.E, self.G = N, E, G
        self.XD, self.ED, self.C, self.L = XD, ED, C, L
        self.ncores = ncores
        self.use_collectives = use_collectives and ncores > 1
        self.use_shared = use_shared
        self.gather_k = gather_k
        self.no_indirect = no_indirect
        self.gather_plain = gather_plain
        self.n_layers = L if n_layers is None else n_layers
        self.TPC = (N + ncores * 128 - 1) // (ncores * 128)    # node tiles per core
        self.NB = self.TPC * 128                               # nodes per core (padded)
        self.NPAD = self.NB * ncores
        self.PG = ((G + 128) + 127) // 128 * 128               # pooled scatter rows
        self.Kg = None                                         # chunks per tile g [TPC]
        self.E_PAD = None                                      # padded edges per core


# ----------------------------------------------------------------------------
# Host-side preprocessing: sort/bucket edges, build per-core input arrays.
# ----------------------------------------------------------------------------

def preprocess(inputs, cfg):
    x = np.ascontiguousarray(np.asarray(inputs["x"], np.float32))
    ea = np.ascontiguousarray(np.asarray(inputs["edge_attr"], np.float32))
    ei = np.asarray(inputs["edge_index"]).astype(np.int64)
    batch = np.asarray(inputs["batch"]).astype(np.int64)
    N, E, G = cfg.N, cfg.E, cfg.G
    XD, ED, C, L = cfg.XD, cfg.ED, cfg.C, cfg.L
    TPC, NB = cfg.TPC, cfg.NB
    NC_ = cfg.ncores

    assert np.abs(x).max() < 16.0, "x out of safe no-clip range"
    assert np.abs(ea).max() < 16.0, "edge_attr out of safe no-clip range"

    src, dst = ei[0], ei[1]
    # sort by (dst tile, src): tile bucketing unchanged, but each 128-edge
    # chunk then gathers ascending clustered h rows (HBM locality)
    order = np.argsort((dst // 128).astype(np.int64) * (N + 1) + src,
                       kind="stable")
    src_s = src[order]
    dst_s = dst[order]
    ea_s = ea[order]

    ntiles = NC_ * TPC
    bnd = np.searchsorted(dst_s, np.arange(ntiles + 1) * 128)
    cnt = np.diff(bnd)
    K_t = np.maximum((cnt + 127) // 128, 1).reshape(NC_, TPC)
    Kg = K_t.max(axis=0).astype(np.int64)                     # [TPC] shared
    E_PAD = int(Kg.sum()) * 128
    Foff = np.concatenate([[0], np.cumsum(Kg) * 128])         # flat slot offset per g

    idx_a = np.zeros((NC_, E_PAD), np.int32)
    dloc_a = np.full((NC_, E_PAD), -1.0, np.float32)
    eaT_a = np.zeros((NC_, ED + 1, E_PAD), np.float32)
    eaT_a[:, ED, :] = 1.0
    for c in range(NC_):
        for g in range(TPC):
            t = c * TPC + g
            m = int(cnt[t])
            if m == 0:
                continue
            K = int(Kg[g])
            f = np.arange(128 * K)
            p, j = f // K, f % K
            es = j * 128 + p                    # edge slot in chunk-major order
            v = es < m
            rows = bnd[t] + es[v]
            fo = int(Foff[g])
            idx_a[c, fo + f[v]] = src_s[rows]
            dloc_a[c, fo + f[v]] = (dst_s[rows] - t * 128).astype(np.float32)
            eaT_a[c, :ED, fo + f[v]] = ea_s[rows]

    xT_a = np.zeros((NC_, XD + 1, NB), np.float32)
    xT_a[:, XD, :] = 1.0
    bloc_a = np.full((NC_, NB), -1.0, np.float32)
    first_g = np.zeros(NC_, np.int64)
    for c in range(NC_):
        lo, hi = c * NB, min((c + 1) * NB, N)
        xT_a[c, :XD, : hi - lo] = x[lo:hi].T
        first_g[c] = batch[lo]
        assert batch[hi - 1] - batch[lo] < 128, "graph window exceeds 128"
        bloc_a[c, : hi - lo] = batch[lo:hi].astype(np.float32)

    prow_a = (first_g[:, None] + np.arange(128)[None, :]).astype(np.int32)
    assert prow_a.max() < cfg.PG
    cnt_g = np.bincount(batch, minlength=G).astype(np.float32)
    cnt_inv = np.zeros(cfg.PG, np.float32)
    cnt_inv[:G] = np.float32(1.0) / np.maximum(cnt_g, np.float32(1.0))

    cfg.Kg = [int(k) for k in Kg]
    cfg.E_PAD = E_PAD

    def f32(a):
        return np.ascontiguousarray(np.asarray(a, np.float32))

    shared = dict(
        W_node=f32(inputs["W_node"]), b_node=f32(inputs["b_node"]).reshape(1, C),
        W_edge=f32(inputs["W_edge"]), b_edge=f32(inputs["b_edge"]).reshape(1, C),
        bnn_g=f32(inputs["bnn_g"]).reshape(1, C), bnn_b=f32(inputs["bnn_b"]).reshape(1, C),
        bnn_m=f32(inputs["bnn_m"]).reshape(1, C), bnn_v=f32(inputs["bnn_v"]).reshape(1, C),
        bne_g=f32(inputs["bne_g"]).reshape(1, C), bne_b=f32(inputs["bne_b"]).reshape(1, C),
        bne_m=f32(inputs["bne_m"]).reshape(1, C), bne_v=f32(inputs["bne_v"]).reshape(1, C),
        t=f32(inputs["t"]).reshape(1, L),
        W1=f32(inputs["W1"]), b1=f32(inputs["b1"]),
        bn1_g=f32(inputs["bn1_g"]), bn1_b=f32(inputs["bn1_b"]),
        bn1_m=f32(inputs["bn1_m"]), bn1_v=f32(inputs["bn1_v"]),
        W2=f32(inputs["W2"]), b2=f32(inputs["b2"]),
        W_out=f32(inputs["W_out"]), b_out=f32(inputs["b_out"]).reshape(1, 1),
        cnt_inv=cnt_inv,
    )
    in_maps = []
    for c in range(NC_):
        im = dict(shared)
        im.update(
            xT=xT_a[c], eaT=eaT_a[c].reshape(-1), gidx=idx_a[c], dloc=dloc_a[c],
            bloc=bloc_a[c], prow=prow_a[c],
        )
        in_maps.append(im)
    return in_maps


# ----------------------------------------------------------------------------
# Device program.
# ----------------------------------------------------------------------------

def emit_q(nc, ap, pre_bias_ap=None, clip=True):
    """In-place fake quantization of `ap` (fp32): q(x) (+fused bias if given).

    If pre_bias_ap is given it must hold (1024*bias_q + MAGIC) per partition and
    the op computes q(x + bias_q)."""
    if pre_bias_ap is None:
        nc.scalar.activation(ap, ap, ACTF.Copy, bias=MAGIC, scale=QS)
    else:
        nc.scalar.activation(ap, ap, ACTF.Identity, bias=pre_bias_ap, scale=QS)
    nc.scalar.activation(ap, ap, ACTF.Copy, bias=QB2, scale=QI)
    if clip:
        nc.vector.tensor_scalar(ap, ap, QMAX, QMIN, AL.min, AL.max)


def build(cfg):
    C, L, TPC, NB = cfg.C, cfg.L, cfg.TPC, cfg.NB
    XD, ED, G, PG = cfg.XD, cfg.ED, cfg.G, cfg.PG
    NPAD, E_PAD, Kg = cfg.NPAD, cfg.E_PAD, cfg.Kg
    C2 = 2 * C
    RG = [list(range(cfg.ncores))]
    SHARED = "Shared" if (cfg.use_shared and cfg.use_collectives) else "Local"

    nc = bacc.Bacc("TRN2", target_bir_lowering=False, debug=False,
                   enable_asserts=False, num_devices=cfg.ncores)

    # ---- kernel I/O ----
    d_xT = nc.dram_tensor("xT", [XD + 1, NB], F32, kind="ExternalInput")
    d_eaT = nc.dram_tensor("eaT", [(ED + 1) * E_PAD], F32, kind="ExternalInput")
    d_gidx = nc.dram_tensor("gidx", [E_PAD], I32, kind="ExternalInput")
    d_dloc = nc.dram_tensor("dloc", [E_PAD], F32, kind="ExternalInput")
    d_bloc = nc.dram_tensor("bloc", [NB], F32, kind="ExternalInput")
    d_prow = nc.dram_tensor("prow", [128], I32, kind="ExternalInput")
    d_cntinv = nc.dram_tensor("cnt_inv", [PG], F32, kind="ExternalInput")
    d_Wn = nc.dram_tensor("W_node", [XD, C], F32, kind="ExternalInput")
    d_bn_ = nc.dram_tensor("b_node", [1, C], F32, kind="ExternalInput")
    d_We = nc.dram_tensor("W_edge", [ED, C], F32, kind="ExternalInput")
    d_be = nc.dram_tensor("b_edge", [1, C], F32, kind="ExternalInput")
    d_bnr = {k: nc.dram_tensor(k, [1, C], F32, kind="ExternalInput")
             for k in ["bnn_g", "bnn_b", "bnn_m", "bnn_v",
                       "bne_g", "bne_b", "bne_m", "bne_v"]}
    d_t = nc.dram_tensor("t", [1, L], F32, kind="ExternalInput")
    d_W1 = nc.dram_tensor("W1", [L, C, C2], F32, kind="ExternalInput")
    d_b1 = nc.dram_tensor("b1", [L, C2], F32, kind="ExternalInput")
    d_bn1 = {k: nc.dram_tensor(k, [L, C2], F32, kind="ExternalInput")
             for k in ["bn1_g", "bn1_b", "bn1_m", "bn1_v"]}
    d_W2 = nc.dram_tensor("W2", [L, C2, C], F32, kind="ExternalInput")
    d_b2 = nc.dram_tensor("b2", [L, C], F32, kind="ExternalInput")
    d_Wo = nc.dram_tensor("W_out", [C, 1], F32, kind="ExternalInput")
    d_bo = nc.dram_tensor("b_out", [1, 1], F32, kind="ExternalInput")
    d_out = nc.dram_tensor("out", [G, 1], F32, kind="ExternalOutput")
    d_hdbg = nc.dram_tensor("h_dbg", [NPAD, C], F32, kind="ExternalOutput")

    # ---- inline constants ----
    eye = np.eye(128, dtype=np.float32)
    iota = np.tile(np.arange(128, dtype=np.float32), (128, 1))
    iota4_np = np.tile(np.arange(128, dtype=np.float32), (128, 4, 1))
    ones_np = np.ones((1, 128), np.float32)
    c_eye = nc.inline_tensor(eye, "c_eye")
    c_iota = nc.inline_tensor(iota, "c_iota")
    c_iota4 = nc.inline_tensor(iota4_np, "c_iota4")
    NW = PG // 128                                   # pooling windows
    iota5_np = (np.tile(np.arange(128, dtype=np.float32), (128, NW, 1))
                + (np.arange(NW, dtype=np.float32) * 128)[None, :, None])
    c_iota5 = nc.inline_tensor(iota5_np, "c_iota5")
    c_ones = nc.inline_tensor(ones_np, "c_ones")

    with tile.TileContext(nc) as tc:
        with (
            tc.tile_pool(name="dram", bufs=1, space="DRAM") as dpool,
            tc.tile_pool(name="const", bufs=1) as cp,
        ):
            # ---- internal DRAM ----
            h_locA = dpool.tile([NB, C], F32, name="h_locA")
            h_locB = dpool.tile([NB, C], F32, name="h_locB")
            h_fulls = [dpool.tile([NPAD, C], F32, addr_space=SHARED,
                                  name=f"h_full_{l}") for l in range(L)]
            e_dram = dpool.tile([E_PAD * C], F32, name="e_dram")
            xq_dram = dpool.tile([(XD + 1) * NB], F32, name="xq_dram")
            eaq_dram = dpool.tile([(ED + 1) * E_PAD], F32, name="eaq_dram")
            pool_glob = dpool.tile([PG, C], F32, name="pool_glob")
            pool_red = dpool.tile([PG, C], F32, addr_space=SHARED, name="pool_red")

            # ---- constants to SBUF ----
            ident = cp.tile([128, 128], F32, name="ident")
            nc.sync.dma_start(ident[:, :], c_eye[:, :])
            iota1 = cp.tile([128, 128], F32, name="iota1")
            nc.sync.dma_start(iota1[:, :], c_iota[:, :])
            iota4 = cp.tile([128, 4, 128], F32, name="iota4")
            nc.sync.dma_start(iota4[:, :, :], c_iota4[:, :, :])
            iota5 = cp.tile([128, NW, 128], F32, name="iota5")
            nc.sync.dma_start(iota5[:, :, :], c_iota5[:, :, :])
            pacc = cp.tile([128, NW, C], F32, name="pacc")
            nc.gpsimd.memset(pacc[:, :, :], 0.0)
            onesr = cp.tile([1, 128], F32, name="onesr")
            nc.sync.dma_start(onesr[:, :], c_ones[:, :])

            # ---- parameter prep ----
            rhs_node = cp.tile([XD + 1, C], F32, name="rhs_node")
            nc.sync.dma_start(rhs_node[:XD, :], d_Wn[:, :])
            nc.sync.dma_start(rhs_node[XD:XD + 1, :], d_bn_[:, :])
            emit_q(nc, rhs_node[:, :])
            rhs_edge = cp.tile([ED + 1, C], F32, name="rhs_edge")
            nc.sync.dma_start(rhs_edge[:ED, :], d_We[:, :])
            nc.sync.dma_start(rhs_edge[ED:ED + 1, :], d_be[:, :])
            emit_q(nc, rhs_edge[:, :])

            def bn_rows(pref):
                g_ = cp.tile([1, C], F32, name=pref + "_g")
                b_ = cp.tile([1, C], F32, name=pref + "_b")
                m_ = cp.tile([1, C], F32, name=pref + "_m")
                sc = cp.tile([1, C], F32, name=pref + "_sc")
                bi = cp.tile([1, C], F32, name=pref + "_bi")
                nc.sync.dma_start(g_[:, :], d_bnr[pref + "_g"][:, :])
                nc.sync.dma_start(b_[:, :], d_bnr[pref + "_b"][:, :])
                nc.sync.dma_start(m_[:, :], d_bnr[pref + "_m"][:, :])
                nc.sync.dma_start(sc[:, :], d_bnr[pref + "_v"][:, :])
                nc.vector.tensor_scalar(sc[:, :], sc[:, :], BN_EPS, None, AL.add)
                nc.scalar.activation(sc[:, :], sc[:, :], ACTF.Sqrt)
                nc.vector.reciprocal(sc[:, :], sc[:, :])
                nc.vector.tensor_tensor(sc[:, :], sc[:, :], g_[:, :], op=AL.mult)
                nc.vector.tensor_tensor(bi[:, :], m_[:, :], sc[:, :], op=AL.mult)
                nc.vector.tensor_tensor(bi[:, :], b_[:, :], bi[:, :], op=AL.subtract)
                return sc, bi

            scN, biN = bn_rows("bnn")
            scE, biE = bn_rows("bne")

            def replicate4(row, nm, pool):
                ps = pool.tile([128, C], F32, name="rep_ps", tag="encp")
                nc.tensor.matmul(ps[:, :], lhsT=onesr[:, :], rhs=row[:, :],
                                 start=True, stop=True)
                out4 = cp.tile([128, 4 * C], F32, name=nm)
                for q in range(4):
                    nc.vector.tensor_copy(out4[:, q * C:(q + 1) * C], ps[:, :])
                return out4

            W1q, bias1, sc1, bi1, W2q, bias2 = [], [], [], [], [], []
            for l in range(L):
                w1 = cp.tile([C, C2], F32, name=f"W1q_{l}")
                nc.sync.dma_start(w1[:, :], d_W1[l, :, :])
                emit_q(nc, w1[:, :])
                W1q.append(w1)
                b1t = cp.tile([C2, 1], F32, name=f"bias1_{l}")
                nc.sync.dma_start(b1t[:, :], d_b1[l:l + 1, :].rearrange("a b -> b a"))
                emit_q(nc, b1t[:, :])
                nc.vector.tensor_scalar(b1t[:, :], b1t[:, :], QS, MAGIC, AL.mult, AL.add)
                bias1.append(b1t)

                g1 = cp.tile([C2, 1], F32, name=f"g1_{l}")
                bb1 = cp.tile([C2, 1], F32, name=f"bb1_{l}")
                m1 = cp.tile([C2, 1], F32, name=f"m1_{l}")
                s1 = cp.tile([C2, 1], F32, name=f"sc1_{l}")
                i1 = cp.tile([C2, 1], F32, name=f"bi1_{l}")
                nc.sync.dma_start(g1[:, :], d_bn1["bn1_g"][l:l + 1, :].rearrange("a b -> b a"))
                nc.sync.dma_start(bb1[:, :], d_bn1["bn1_b"][l:l + 1, :].rearrange("a b -> b a"))
                nc.sync.dma_start(m1[:, :], d_bn1["bn1_m"][l:l + 1, :].rearrange("a b -> b a"))
                nc.sync.dma_start(s1[:, :], d_bn1["bn1_v"][l:l + 1, :].rearrange("a b -> b a"))
                nc.vector.tensor_scalar(s1[:, :], s1[:, :], BN_EPS, None, AL.add)
                nc.scalar.activation(s1[:, :], s1[:, :], ACTF.Sqrt)
                nc.vector.reciprocal(s1[:, :], s1[:, :])
                nc.vector.tensor_tensor(s1[:, :], s1[:, :], g1[:, :], op=AL.mult)
                nc.vector.tensor_tensor(i1[:, :], m1[:, :], s1[:, :], op=AL.mult)
                nc.vector.tensor_tensor(i1[:, :], bb1[:, :], i1[:, :], op=AL.subtract)
                sc1.append(s1)
                bi1.append(i1)

                w2 = cp.tile([C2, C], F32, name=f"W2q_{l}")
                nc.sync.dma_start(w2[:, :], d_W2[l, :, :])
                emit_q(nc, w2[:, :])
                W2q.append(w2)
                b2t = cp.tile([C, 1], F32, name=f"bias2_{l}")
                nc.sync.dma_start(b2t[:, :], d_b2[l:l + 1, :].rearrange("a b -> b a"))
                emit_q(nc, b2t[:, :])
                nc.vector.tensor_scalar(b2t[:, :], b2t[:, :], QS, MAGIC, AL.mult, AL.add)
                bias2.append(b2t)

            Woq = cp.tile([C, 1], F32, name="Woq")
            nc.sync.dma_start(Woq[:, :], d_Wo[:, :])
            emit_q(nc, Woq[:, :])
            biaso = cp.tile([1, 1], F32, name="biaso")
            nc.sync.dma_start(biaso[:, :], d_bo[:, :])
            emit_q(nc, biaso[:, :])
            nc.vector.tensor_scalar(biaso[:, :], biaso[:, :], QS, MAGIC, AL.mult, AL.add)


            # ---- encoders ----
            def q_pass(src_flat, dst_flat, total, pool):
                per = total // 128
                assert total % 128 == 0
                W = min(per, 4096)
                n = (per + W - 1) // W
                sv = src_flat.rearrange("(p q) -> p q", p=128)
                dv = dst_flat.rearrange("(p q) -> p q", p=128)
                for i in range(n):
                    w = min(W, per - i * W)
                    tl = pool.tile([128, W], F32, tag="qpass", name="qpass")
                    nc.sync.dma_start(tl[:, :w], sv[:, i * W:i * W + w])
                    nc.scalar.activation(tl[:, :w], tl[:, :w], ACTF.Copy,
                                         bias=MAGIC, scale=QS)
                    nc.scalar.activation(tl[:, :w], tl[:, :w], ACTF.Copy,
                                         bias=QB2, scale=QI)
                    nc.sync.dma_start(dv[:, i * W:i * W + w], tl[:, :w])

            with (
                tc.tile_pool(name="enc", bufs=2) as enc,
                tc.tile_pool(name="encx", bufs=1) as encx,
                tc.tile_pool(name="encps", bufs=2, space="PSUM") as enc_ps,
            ):
                scN4 = replicate4(scN, "scN4", enc_ps)
                biN4 = replicate4(biN, "biN4", enc_ps)
                scE4 = replicate4(scE, "scE4", enc_ps)
                biE4 = replicate4(biE, "biE4", enc_ps)

                t_sb = cp.tile([1, L], F32, name="t_sb")
                nc.sync.dma_start(t_sb[:, :], d_t[:, :])
                t_ps = enc_ps.tile([128, L], F32, name="t_ps", tag="encp")
                nc.tensor.matmul(t_ps[:, :], lhsT=onesr[:, :], rhs=t_sb[:, :],
                                 start=True, stop=True)
                t_bc = cp.tile([128, L], F32, name="t_bc")
                nc.vector.tensor_copy(t_bc[:, :], t_ps[:, :])
                teps_bc = cp.tile([128, L], F32, name="teps_bc")
                nc.vector.tensor_scalar(teps_bc[:, :], t_bc[:, :], GEN_EPS, None,
                                        AL.mult)

                # node encoder
                q_pass(d_xT[:, :].rearrange("a b -> (a b)"), xq_dram[:], (XD + 1) * NB, enc)
                xseg = encx.tile([XD + 1, NB], F32, name="xseg")
                nc.sync.dma_start(
                    xseg[:, :], xq_dram[:].rearrange("(r e) -> r e", r=XD + 1))
                for b in range(0, TPC, 4):
                    gs = min(4, TPC - b)
                    ep = enc_ps.tile([128, 4 * C], F32, name="encp", tag="encp")
                    for q in range(gs):
                        nc.tensor.matmul(
                            ep[:, q * C:(q + 1) * C],
                            lhsT=xseg[:, (b + q) * 128:(b + q + 1) * 128],
                            rhs=rhs_node[:, :], start=True, stop=True)
                    es = enc.tile([128, 4 * C], F32, name="encs", tag="encs")
                    nc.scalar.activation(es[:, :gs * C], ep[:, :gs * C], ACTF.Copy,
                                         bias=MAGIC, scale=QS)
                    nc.scalar.activation(es[:, :gs * C], es[:, :gs * C], ACTF.Copy,
                                         bias=QB2, scale=QI)
                    nc.vector.tensor_scalar(es[:, :gs * C], es[:, :gs * C],
                                            QMAX, QMIN, AL.min, AL.max)
                    nc.vector.tensor_tensor(es[:, :gs * C], es[:, :gs * C],
                                            scN4[:, :gs * C], op=AL.mult)
                    nc.vector.tensor_tensor(es[:, :gs * C], es[:, :gs * C],
                                            biN4[:, :gs * C], op=AL.add)
                    nc.sync.dma_start(
                        h_locA[b * 128:(b + gs) * 128, :]
                        .rearrange("(t p) c -> p t c", p=128),
                        es[:, :gs * C].rearrange("p (t c) -> p t c", c=C))

                # first AllGather (before the edge encoder so its latency
                # hides behind ~1.7ms of edge-encoder work)
                if cfg.use_collectives:
                    nc.gpsimd.collective_compute(
                        "AllGather", AL.bypass, replica_groups=RG,
                        ins=[h_locA[:, :]], outs=[h_fulls[0][:, :]])
                else:
                    for b_ in range(cfg.ncores):
                        nc.sync.dma_start(h_fulls[0][b_ * NB:(b_ + 1) * NB, :],
                                          h_locA[:, :])

                # edge encoder
                q_pass(d_eaT[:], eaq_dram[:], (ED + 1) * E_PAD, enc)
                eav = eaq_dram[:].rearrange("(r e) -> r e", r=ED + 1)
                n_ch = E_PAD // 128
                SEGC = 32                       # chunks per staged segment
                for s0 in range(0, n_ch, SEGC):
                    sc_ = min(SEGC, n_ch - s0)
                    eseg = enc.tile([ED + 1, SEGC * 128], F32, name="eseg", tag="eseg")
                    nc.sync.dma_start(eseg[:, :sc_ * 128],
                                      eav[:, s0 * 128:(s0 + sc_) * 128])
                    for b in range(0, sc_, 4):
                        gs = min(4, sc_ - b)
                        ep = enc_ps.tile([128, 4 * C], F32, name="encp", tag="encp")
                        for q in range(gs):
                            nc.tensor.matmul(
                                ep[:, q * C:(q + 1) * C],
                                lhsT=eseg[:, (b + q) * 128:(b + q + 1) * 128],
                                rhs=rhs_edge[:, :], start=True, stop=True)
                        es = enc.tile([128, 4 * C], F32, name="encs2", tag="encs")
                        nc.scalar.activation(es[:, :gs * C], ep[:, :gs * C], ACTF.Copy,
                                             bias=MAGIC, scale=QS)
                        nc.scalar.activation(es[:, :gs * C], es[:, :gs * C], ACTF.Copy,
                                             bias=QB2, scale=QI)
                        nc.vector.tensor_scalar(es[:, :gs * C], es[:, :gs * C],
                                                QMAX, QMIN, AL.min, AL.max)
                        nc.vector.tensor_tensor(es[:, :gs * C], es[:, :gs * C],
                                                scE4[:, :gs * C], op=AL.mult)
                        nc.vector.tensor_tensor(es[:, :gs * C], es[:, :gs * C],
                                                biE4[:, :gs * C], op=AL.add)
                        r0 = (s0 + b) * 128
                        nc.sync.dma_start(
                            e_dram[r0 * C:(r0 + gs * 128) * C]
                            .rearrange("(t p c) -> p t c", p=128, c=C),
                            es[:, :gs * C].rearrange("p (t c) -> p t c", c=C))


            # ---- layers ----
            with (
                tc.tile_pool(name="edge", bufs=3) as epool,
                tc.tile_pool(name="node", bufs=2) as npool,
                tc.tile_pool(name="eps", bufs=2, space="PSUM") as ps_edge,
                tc.tile_pool(name="mlp1", bufs=2, space="PSUM") as ps_z1,
                tc.tile_pool(name="mlp2", bufs=1, space="PSUM") as ps_z2,
                tc.tile_pool(name="tr", bufs=2, space="PSUM") as ps_tr,
                tc.tile_pool(name="poolps", bufs=1, space="PSUM") as ps_pool,
            ):
              Foff = np.concatenate([[0], np.cumsum(np.asarray(Kg)) * 128]).astype(int)

              for l in range(min(L, cfg.n_layers)):
                  h_in = h_locA if l % 2 == 0 else h_locB
                  h_out = h_locB if l % 2 == 0 else h_locA
                  last = l == L - 1

                  hog = None
                  h2qT = None
                  for g in range(TPC):
                      K = Kg[g]
                      F = int(Foff[g])
                      t = g % 4
                      if t == 0:
                          gs = min(4, TPC - g)
                          hog = npool.tile([128, 4, C], F32, name="hog", tag="hog")
                          h2qT = npool.tile([C, 512], F32, name="h2qT", tag="h2qT")

                      # --- edge phase ---
                      et = epool.tile([128, K, C], F32, name="et", tag="et",
                                      padded_shape=[128, max(Kg), C])
                      nc.sync.dma_start(
                          et[:, :, :],
                          e_dram[F * C:(F + 128 * K) * C]
                          .rearrange("(p k c) -> p k c", p=128, k=K))
                      idxt = epool.tile([128, K], I32, name="idxt", tag="idxt",
                                        padded_shape=[128, max(Kg)])
                      nc.sync.dma_start(
                          idxt[:, :],
                          d_gidx[F:F + 128 * K].rearrange("(p k) -> p k", p=128))
                      dlt = epool.tile([128, K], F32, name="dlt", tag="dlt",
                                       padded_shape=[128, max(Kg)])
                      nc.sync.dma_start(
                          dlt[:, :],
                          d_dloc[F:F + 128 * K].rearrange("(p k) -> p k", p=128))
                      # gather h[src] and accumulate onto e -> et = h_src + e
                      # (HW indirect DMA consumes ONE index per partition per
                      #  instruction, so gather chunk-by-chunk: [128,1] idx.)
                      if not cfg.no_indirect:
                          if cfg.gather_plain:
                              hsg = epool.tile([128, K, C], F32, name="hsg",
                                               tag="hsg",
                                               padded_shape=[128, max(Kg), C])
                              for j in range(K):
                                  nc.gpsimd.indirect_dma_start(
                                      out=hsg[:, j, :], out_offset=None,
                                      in_=h_fulls[l][:, :],
                                      in_offset=IndirectOffsetOnAxis(
                                          ap=idxt[:, j:j + 1], axis=0))
                              nc.vector.tensor_tensor(et[:, :, :], et[:, :, :],
                                                      hsg[:, :, :], op=AL.add)
                          else:
                              for j in range(K):
                                  nc.gpsimd.indirect_dma_start(
                                      out=et[:, j, :], out_offset=None,
                                      in_=h_fulls[l][:, :],
                                      in_offset=IndirectOffsetOnAxis(
                                          ap=idxt[:, j:j + 1], axis=0),
                                      compute_op=AL.add)
                      # r = relu(h_src + e)
                      nc.scalar.activation(et[:, :, :], et[:, :, :], ACTF.Relu)
                      exmex = epool.tile([128, K, C2], F32, name="exmex", tag="exmex",
                                         padded_shape=[128, max(Kg), C2])
                      # ex = exp(t_l * r + t_l*eps)
                      nc.scalar.activation(exmex[:, :, C:C2], et[:, :, :], ACTF.Exp,
                                           bias=teps_bc[:, l:l + 1],
                                           scale=t_bc[:, l:l + 1])
                      # m = r + eps
                      nc.vector.tensor_scalar(et[:, :, :], et[:, :, :], GEN_EPS,
                                              None, AL.add)
                      # ex*m
                      nc.vector.tensor_tensor(exmex[:, :, 0:C], exmex[:, :, C:C2],
                                              et[:, :, :], op=AL.mult)

                      eps_t = ps_edge.tile([128, C2], F32, name="eps_t", tag="eps_t")
                      for j0 in range(0, K, 4):
                          jj = min(4, K - j0)
                          oh4 = epool.tile([128, 4, 128], F32, name="oh4", tag="oh4")
                          nc.vector.tensor_tensor(
                              oh4[:, :jj, :],
                              dlt[:, j0:j0 + jj].to_broadcast([128, jj, 128]),
                              iota4[:, :jj, :], op=AL.is_equal)
                          for q in range(jj):
                              j = j0 + q
                              nc.tensor.matmul(
                                  eps_t[:, :], lhsT=oh4[:, q, :],
                                  rhs=exmex[:, j, :],
                                  start=(j == 0), stop=(j == K - 1))

                      # agg = num / max(den, 1e-16);  h2 = h_own + agg
                      nc.sync.dma_start(hog[:, t, :], h_in[g * 128:(g + 1) * 128, :])
                      dinv = npool.tile([128, C], F32, name="dinv", tag="dinv")
                      nc.vector.tensor_scalar(dinv[:, :], eps_t[:, C:C2], 1e-16,
                                              None, AL.max)
                      nc.vector.reciprocal(dinv[:, :], dinv[:, :])
                      h2 = npool.tile([128, C], F32, name="h2", tag="h2")
                      nc.vector.tensor_tensor(h2[:, :], eps_t[:, 0:C], dinv[:, :],
                                              op=AL.mult)
                      nc.vector.tensor_tensor(h2[:, :], h2[:, :], hog[:, t, :],
                                              op=AL.add)
                      emit_q(nc, h2[:, :])
                      trp = ps_tr.tile([C, 128], F32, name="trp", tag="tr")
                      nc.tensor.transpose(trp[:, :], h2[:, :], identity=ident[:, :])
                      nc.vector.tensor_copy(h2qT[:, t * 128:(t + 1) * 128], trp[:, :])

                      # --- MLP on a full group of up to 4 node tiles ---
                      if t == gs - 1 or g == TPC - 1:
                          w = gs * 128
                          z1p = ps_z1.tile([C2, 512], F32, name="z1p", tag="z1p")
                          nc.tensor.matmul(z1p[:, :w], lhsT=W1q[l][:, :],
                                           rhs=h2qT[:, :w], start=True, stop=True)
                          z1s = npool.tile([C2, 512], F32, name="z1s", tag="z1s")
                          nc.scalar.activation(z1s[:, :w], z1p[:, :w], ACTF.Identity,
                                               bias=bias1[l][:, :], scale=QS)
                          nc.scalar.activation(z1s[:, :w], z1s[:, :w], ACTF.Copy,
                                               bias=QB2, scale=QI)
                          nc.vector.tensor_scalar(z1s[:, :w], z1s[:, :w], QMAX, QMIN,
                                                  AL.min, AL.max)
                          nc.scalar.activation(z1s[:, :w], z1s[:, :w], ACTF.Relu,
                                               bias=bi1[l][:, :], scale=sc1[l][:, :])
                          nc.scalar.activation(z1s[:, :w], z1s[:, :w], ACTF.Copy,
                                               bias=MAGIC, scale=QS)
                          nc.scalar.activation(z1s[:, :w], z1s[:, :w], ACTF.Copy,
                                               bias=QB2, scale=QI)
                          nc.vector.tensor_scalar(z1s[:, :w], z1s[:, :w], QMAX, QMIN,
                                                  AL.min, AL.max)
                          z2p = ps_z2.tile([C, 512], F32, name="z2p", tag="z2p")
                          nc.tensor.matmul(z2p[:, :w], lhsT=W2q[l][:, :],
                                           rhs=z1s[:, :w], start=True, stop=True)
                          z2s = npool.tile([C, 512], F32, name="z2s", tag="z2s")
                          nc.scalar.activation(z2s[:, :w], z2p[:, :w], ACTF.Identity,
                                               bias=bias2[l][:, :], scale=QS)
                          nc.scalar.activation(z2s[:, :w], z2s[:, :w], ACTF.Copy,
                                               bias=QB2, scale=QI)
                          nc.vector.tensor_scalar(z2s[:, :w], z2s[:, :w], QMAX, QMIN,
                                                  AL.min, AL.max)
                          hnext = npool.tile([128, 4, C], F32, name="hnext", tag="hnext")
                          g0 = g - gs + 1
                          for q in range(gs):
                              trq = ps_tr.tile([128, C], F32, name="trq", tag="tr")
                              nc.tensor.transpose(trq[:, :],
                                                  z2s[:, q * 128:(q + 1) * 128],
                                                  identity=ident[0:C, 0:C])
                              nc.vector.tensor_tensor(hnext[:, q, :], trq[:, :],
                                                      hog[:, q, :], op=AL.add)
                              if last:
                                  blt = npool.tile([128, 1], F32, name="blt", tag="blt")
                                  nc.sync.dma_start(
                                      blt[:, :],
                                      d_bloc[(g0 + q) * 128:(g0 + q + 1) * 128]
                                      .rearrange("(p one) -> p one", one=1))
                                  ohp = npool.tile([128, NW, 128], F32, name="ohp",
                                                   tag="ohp")
                                  nc.vector.tensor_tensor(
                                      ohp[:, :, :],
                                      blt[:, :].to_broadcast([128, NW, 128]),
                                      iota5[:, :, :], op=AL.is_equal)
                                  for wi in range(NW):
                                      pps = ps_pool.tile([128, C], F32, name="pps",
                                                         tag="pps")
                                      nc.tensor.matmul(
                                          pps[:, :], lhsT=ohp[:, wi, :],
                                          rhs=hnext[:, q, :],
                                          start=True, stop=True)
                                      nc.vector.tensor_tensor(
                                          pacc[:, wi, :], pacc[:, wi, :], pps[:, :],
                                          op=AL.add)
                          if not last:
                              nc.sync.dma_start(
                                  h_out[g0 * 128:(g0 + gs) * 128, :]
                                  .rearrange("(t p) c -> p t c", p=128),
                                  hnext[:, :gs, :])

                  if not last:
                      if cfg.use_collectives:
                          nc.gpsimd.collective_compute(
                              "AllGather", AL.bypass, replica_groups=RG,
                              ins=[h_out[:, :]], outs=[h_fulls[l + 1][:, :]])
                      else:
                          for b_ in range(cfg.ncores):
                              nc.sync.dma_start(
                                  h_fulls[l + 1][b_ * NB:(b_ + 1) * NB, :],
                                  h_out[:, :])

              if cfg.n_layers < L:
                  nl = cfg.n_layers
                  hf = h_fulls[min(nl, L - 1)]
                  for b_ in range(NPAD // 128):
                      dbg_t = npool.tile([128, C], F32, name="dbg_t", tag="dbg_t")
                      nc.sync.dma_start(dbg_t[:, :],
                                        hf[b_ * 128:(b_ + 1) * 128, :])
                      nc.sync.dma_start(d_hdbg[b_ * 128:(b_ + 1) * 128, :],
                                        dbg_t[:, :])
                  return nc

              # ---- pooling: write window partials, AllReduce, output head ----
              nc.sync.dma_start(
                  pool_glob[:, :].rearrange("(w p) c -> p w c", p=128),
                  pacc[:, :, :])
              if cfg.use_collectives:
                  nc.gpsimd.collective_compute(
                      "AllReduce", AL.add, replica_groups=RG,
                      ins=[pool_glob[:, :]], outs=[pool_red[:, :]])
              else:
                  nc.sync.dma_start(pool_red[:, :], pool_glob[:, :])

              n_out_tiles = (G + 127) // 128
              for i in range(n_out_tiles):
                  w = min(128, G - i * 128)
                  pt = npool.tile([128, C], F32, name="pt", tag="pt")
                  nc.sync.dma_start(pt[:w, :], pool_red[i * 128:i * 128 + w, :])
                  civ = npool.tile([128, 1], F32, name="civ", tag="civ")
                  nc.sync.dma_start(civ[:w, :],
                                    d_cntinv[i * 128:i * 128 + w].rearrange("(p one) -> p one", one=1))
                  nc.vector.tensor_scalar(pt[:w, :], pt[:w, :], civ[:w, :], None, AL.mult)
                  emit_q(nc, pt[:w, :])
                  trh = ps_tr.tile([C, 128], F32, name="trh", tag="tr")
                  nc.tensor.transpose(trh[:, :w], pt[:w, :], identity=ident[:w, :w])
                  hts = npool.tile([C, 128], F32, name="hts", tag="hts")
                  nc.vector.tensor_copy(hts[:, :w], trh[:, :w])
                  op_ = ps_z2.tile([1, 128], F32, name="op_", tag="z2p")
                  nc.tensor.matmul(op_[:, :w], lhsT=Woq[:, :], rhs=hts[:, :w],
                                   start=True, stop=True)
                  osb = npool.tile([1, 128], F32, name="osb", tag="osb")
                  nc.scalar.activation(osb[:, :w], op_[:, :w], ACTF.Identity,
                                       bias=biaso[:, :], scale=QS)
                  nc.scalar.activation(osb[:, :w], osb[:, :w], ACTF.Copy,
                                       bias=QB2, scale=QI)
                  nc.vector.tensor_scalar(osb[:, :w], osb[:, :w], QMAX, QMIN,
                                          AL.min, AL.max)
                  nc.scalar.activation(osb[:, :w], osb[:, :w], ACTF.Sigmoid)
                  nc.scalar.activation(osb[:, :w], osb[:, :w], ACTF.Copy,
                                       bias=MAGIC, scale=QS)
                  nc.scalar.activation(osb[:, :w], osb[:, :w], ACTF.Copy,
                                       bias=QB2, scale=QI)
                  nc.sync.dma_start(
                      d_out[i * 128:i * 128 + w, :].rearrange("w one -> one w"),
                      osb[:, :w])

    return nc


# ----------------------------------------------------------------------------
# Entry point.
# ----------------------------------------------------------------------------

def run(inputs, cfg, **run_kwargs):
    global LAST_RESULTS
    in_maps = preprocess(inputs, cfg)
    nc = build(cfg)
    if not nc.is_finalized():
        nc.finalize()
    res = run_bass_kernel_spmd(nc, in_maps, core_ids=list(range(cfg.ncores)),
                               **run_kwargs)
    LAST_RESULTS = res
    return res.results[0]["out"].reshape(cfg.G, 1).astype(np.float32)


def kernel(**inputs) -> np.ndarray:
    cfg = Cfg(N=100000, E=3200000, G=512, XD=8, ED=4, C=32, L=4)
    return run(inputs, cfg)



# revision 11
# speedup vs baseline: 1.1824x; 1.1824x over previous
"""Trainium2 Bass kernel: nn_BV_Model (GENConv GNN, softmax aggregation, 4 layers).

Strategy (8 NeuronCores, SPMD):
  - Nodes are partitioned into 8 contiguous blocks (12544/core, padded).
  - Edges are sorted by destination node and bucketed per destination
    node-tile (128 nodes); each core owns the edges whose dst falls in its
    block.  Tiles are processed in groups of 4; within a group the edge
    slots are laid out "group-flat" [p][kk][c] (p = SBUF partition,
    kk = chunk slot within the group) so each group's edge features /
    src indices / dst offsets load as one large DMA and h[src] is fetched
    with ONE batched indirect DMA (~13k descriptors) per group.
  - Per layer: m = relu(h[src]+e), ex = exp(t*m) (bf16), and the segment
    softmax numerator/denominator reduce edges->nodes with one-hot(dst)
    matmuls (bf16) accumulated in PSUM.  No segment-max: s_max ~ 65,
    exp fits fp32/bf16 range (verified offline).  Node MLP runs on the
    tensor engine in transposed layout.  h is AllGathered per layer.
  - Global mean pool via one-hot(graph) matmuls, AllReduce, output head.

Fake-quantization q(x) = clip(rne(x*1024), -32768, 32767)/1024 is computed
exactly with the round-to-nearest-even "magic number" trick (+1.5*2^23).
x/edge_attr are pre-quantized on the host (same RNE semantics).
"""

import os
os.environ.setdefault("MYCRO_LOCAL_CACHE", "1")

import math
import numpy as np
import ml_dtypes

import concourse.bacc as bacc
import concourse.tile as tile
import concourse.bass as bass
from concourse import mybir
from concourse.bass import IndirectOffsetOnAxis
from concourse.bass_utils import run_bass_kernel_spmd

F32 = mybir.dt.float32
BF16 = mybir.dt.bfloat16
I32 = mybir.dt.int32
ACTF = mybir.ActivationFunctionType
AL = mybir.AluOpType

MAGIC = 12582912.0           # 1.5*2^23 : fp32 RNE rounding magic
QS = 1024.0                  # 2^10
QI = 1.0 / 1024.0
QB2 = -12288.0               # -MAGIC * 2^-10
QMAX = 32767.0 / 1024.0
QMIN = -32.0
GEN_EPS = 1e-7
BN_EPS = 1e-5
NCORES = 8
GRP = 4                      # node tiles per group (shared w/ MLP batching)

LAST_RESULTS = None          # BassKernelResults of the most recent run (for test.py)


class Cfg:
    def __init__(self, N, E, G, XD=8, ED=4, C=32, L=4, ncores=NCORES,
                 use_collectives=True, use_shared=True,
                 gather_acc=False, gather_max_cols=64, n_layers=None):
        self.N, self.E, self.G = N, E, G
        self.XD, self.ED, self.C, self.L = XD, ED, C, L
        self.ncores = ncores
        self.use_collectives = use_collectives and ncores > 1
        self.use_shared = use_shared
        self.gather_acc = gather_acc
        self.gather_max_cols = gather_max_cols
        self.n_layers = L if n_layers is None else n_layers
        self.TPC = (N + ncores * 128 - 1) // (ncores * 128)    # node tiles per core
        self.NB = self.TPC * 128                               # nodes per core (padded)
        self.NPAD = self.NB * ncores
        self.PG = ((G + 128) + 127) // 128 * 128               # pooled scatter rows
        self.Kg = None                                         # chunks per tile g [TPC]
        self.SKg = None                                        # chunks per group [NG]
        self.E_PAD = None                                      # padded edges per core


def qnp(a):
    """Host-side ap_fixed<16,6> fake quantization (RNE, matches HW magic)."""
    y = np.round(a.astype(np.float64) * QS) * QI
    return np.clip(y, QMIN, QMAX).astype(np.float32)


# ----------------------------------------------------------------------------
# Host-side preprocessing: sort/bucket edges, build per-core input arrays.
# ----------------------------------------------------------------------------

def preprocess(inputs, cfg):
    x = qnp(np.asarray(inputs["x"], np.float32))
    ea = qnp(np.asarray(inputs["edge_attr"], np.float32))
    ei = np.asarray(inputs["edge_index"]).astype(np.int64)
    batch = np.asarray(inputs["batch"]).astype(np.int64)
    N, E, G = cfg.N, cfg.E, cfg.G
    XD, ED, C, L = cfg.XD, cfg.ED, cfg.C, cfg.L
    TPC, NB = cfg.TPC, cfg.NB
    NC_ = cfg.ncores

    src, dst = ei[0], ei[1]
    # sort by (dst tile, src): tile bucketing unchanged, but edges within a
    # tile then gather ascending clustered h rows (HBM locality)
    order = np.argsort((dst // 128).astype(np.int64) * (N + 1) + src,
                       kind="stable")
    src_s = src[order]
    dst_s = dst[order]
    ea_s = ea[order]

    ntiles = NC_ * TPC
    bnd = np.searchsorted(dst_s, np.arange(ntiles + 1) * 128)
    cnt = np.diff(bnd)
    K_t = np.maximum((cnt + 127) // 128, 1).reshape(NC_, TPC)
    Kg = K_t.max(axis=0).astype(np.int64)                     # [TPC] shared
    NG = (TPC + GRP - 1) // GRP
    SKg = np.array([int(Kg[g0:g0 + GRP].sum())
                    for g0 in range(0, TPC, GRP)], np.int64)  # [NG]
    E_PAD = int(Kg.sum()) * 128
    GFoff = np.concatenate([[0], np.cumsum(SKg) * 128])       # flat slot per group

    # Group-flat slot layout: slot(G, t, j, p) = GFoff[G] + p*SK + off_t + j
    idx_a = np.zeros((NC_, E_PAD), np.int32)
    dloc_a = np.full((NC_, E_PAD), -1.0, ml_dtypes.bfloat16)
    eaT_a = np.zeros((NC_, ED + 1, E_PAD), np.float32)
    eaT_a[:, ED, :] = 1.0
    p_ar = np.arange(128)
    for c in range(NC_):
        for Gi in range(NG):
            g0 = Gi * GRP
            gs = min(GRP, TPC - g0)
            SK = int(SKg[Gi])
            base = int(GFoff[Gi])
            off_t = 0
            for t in range(gs):
                g = g0 + t
                tl = c * TPC + g
                m = int(cnt[tl])
                K = int(Kg[g])
                if m > 0:
                    # within-tile edge slot es = j*128 + p  (sorted order)
                    es = (np.arange(K)[None, :] * 128 + p_ar[:, None])  # [128,K]
                    v = es < m
                    rows = bnd[tl] + es[v]
                    flat = base + (p_ar[:, None] * SK + off_t +
                                   np.arange(K)[None, :])[v]
                    idx_a[c, flat] = src_s[rows]
                    dloc_a[c, flat] = (dst_s[rows] - tl * 128).astype(np.float32)
                    # NB: mixed basic/advanced indexing puts the advanced
                    # (flat) axis FIRST in the result -> assign [n, ED]
                    eaT_a[c, :ED, flat] = ea_s[rows]
                off_t += K

    xT_a = np.zeros((NC_, XD + 1, NB), np.float32)
    xT_a[:, XD, :] = 1.0
    bloc_a = np.full((NC_, NB), -1.0, np.float32)
    first_g = np.zeros(NC_, np.int64)
    for c in range(NC_):
        lo, hi = c * NB, min((c + 1) * NB, N)
        xT_a[c, :XD, : hi - lo] = x[lo:hi].T
        first_g[c] = batch[lo]
        assert batch[hi - 1] - batch[lo] < 128, "graph window exceeds 128"
        bloc_a[c, : hi - lo] = batch[lo:hi].astype(np.float32)

    prow_a = (first_g[:, None] + np.arange(128)[None, :]).astype(np.int32)
    assert prow_a.max() < cfg.PG
    cnt_g = np.bincount(batch, minlength=G).astype(np.float32)
    cnt_inv = np.zeros(cfg.PG, np.float32)
    cnt_inv[:G] = np.float32(1.0) / np.maximum(cnt_g, np.float32(1.0))

    cfg.Kg = [int(k) for k in Kg]
    cfg.SKg = [int(k) for k in SKg]
    cfg.E_PAD = E_PAD

    def f32(a):
        return np.ascontiguousarray(np.asarray(a, np.float32))

    shared = dict(
        W_node=f32(inputs["W_node"]), b_node=f32(inputs["b_node"]).reshape(1, C),
        W_edge=f32(inputs["W_edge"]), b_edge=f32(inputs["b_edge"]).reshape(1, C),
        bnn_g=f32(inputs["bnn_g"]).reshape(1, C), bnn_b=f32(inputs["bnn_b"]).reshape(1, C),
        bnn_m=f32(inputs["bnn_m"]).reshape(1, C), bnn_v=f32(inputs["bnn_v"]).reshape(1, C),
        bne_g=f32(inputs["bne_g"]).reshape(1, C), bne_b=f32(inputs["bne_b"]).reshape(1, C),
        bne_m=f32(inputs["bne_m"]).reshape(1, C), bne_v=f32(inputs["bne_v"]).reshape(1, C),
        t=f32(inputs["t"]).reshape(1, L),
        W1=f32(inputs["W1"]), b1=f32(inputs["b1"]),
        bn1_g=f32(inputs["bn1_g"]), bn1_b=f32(inputs["bn1_b"]),
        bn1_m=f32(inputs["bn1_m"]), bn1_v=f32(inputs["bn1_v"]),
        W2=f32(inputs["W2"]), b2=f32(inputs["b2"]),
        W_out=f32(inputs["W_out"]), b_out=f32(inputs["b_out"]).reshape(1, 1),
        cnt_inv=cnt_inv,
    )
    in_maps = []
    for c in range(NC_):
        im = dict(shared)
        im.update(
            xT=xT_a[c], eaT=eaT_a[c].reshape(-1), gidx=idx_a[c], dloc=dloc_a[c],
            bloc=bloc_a[c], prow=prow_a[c],
        )
        in_maps.append(im)
    return in_maps


# ----------------------------------------------------------------------------
# Device program.
# ----------------------------------------------------------------------------

def emit_q(nc, ap, pre_bias_ap=None, clip=True):
    """In-place fake quantization of `ap` (fp32): q(x) (+fused bias if given).

    If pre_bias_ap is given it must hold (1024*bias_q + MAGIC) per partition and
    the op computes q(x + bias_q)."""
    if pre_bias_ap is None:
        nc.scalar.activation(ap, ap, ACTF.Copy, bias=MAGIC, scale=QS)
    else:
        nc.scalar.activation(ap, ap, ACTF.Identity, bias=pre_bias_ap, scale=QS)
    nc.scalar.activation(ap, ap, ACTF.Copy, bias=QB2, scale=QI)
    if clip:
        nc.vector.tensor_scalar(ap, ap, QMAX, QMIN, AL.min, AL.max)


def build(cfg):
    C, L, TPC, NB = cfg.C, cfg.L, cfg.TPC, cfg.NB
    XD, ED, G, PG = cfg.XD, cfg.ED, cfg.G, cfg.PG
    NPAD, E_PAD, Kg, SKg = cfg.NPAD, cfg.E_PAD, cfg.Kg, cfg.SKg
    C2 = 2 * C
    NG = (TPC + GRP - 1) // GRP
    SKMAX = max(SKg)
    RG = [list(range(cfg.ncores))]
    SHARED = "Shared" if (cfg.use_shared and cfg.use_collectives) else "Local"

    nc = bacc.Bacc("TRN2", target_bir_lowering=False, debug=False,
                   enable_asserts=False, num_devices=cfg.ncores)

    # ---- kernel I/O ----
    d_xT = nc.dram_tensor("xT", [XD + 1, NB], F32, kind="ExternalInput")
    d_eaT = nc.dram_tensor("eaT", [(ED + 1) * E_PAD], F32, kind="ExternalInput")
    d_gidx = nc.dram_tensor("gidx", [E_PAD], I32, kind="ExternalInput")
    d_dloc = nc.dram_tensor("dloc", [E_PAD], BF16, kind="ExternalInput")
    d_bloc = nc.dram_tensor("bloc", [NB], F32, kind="ExternalInput")
    d_prow = nc.dram_tensor("prow", [128], I32, kind="ExternalInput")
    d_cntinv = nc.dram_tensor("cnt_inv", [PG], F32, kind="ExternalInput")
    d_Wn = nc.dram_tensor("W_node", [XD, C], F32, kind="ExternalInput")
    d_bn_ = nc.dram_tensor("b_node", [1, C], F32, kind="ExternalInput")
    d_We = nc.dram_tensor("W_edge", [ED, C], F32, kind="ExternalInput")
    d_be = nc.dram_tensor("b_edge", [1, C], F32, kind="ExternalInput")
    d_bnr = {k: nc.dram_tensor(k, [1, C], F32, kind="ExternalInput")
             for k in ["bnn_g", "bnn_b", "bnn_m", "bnn_v",
                       "bne_g", "bne_b", "bne_m", "bne_v"]}
    d_t = nc.dram_tensor("t", [1, L], F32, kind="ExternalInput")
    d_W1 = nc.dram_tensor("W1", [L, C, C2], F32, kind="ExternalInput")
    d_b1 = nc.dram_tensor("b1", [L, C2], F32, kind="ExternalInput")
    d_bn1 = {k: nc.dram_tensor(k, [L, C2], F32, kind="ExternalInput")
             for k in ["bn1_g", "bn1_b", "bn1_m", "bn1_v"]}
    d_W2 = nc.dram_tensor("W2", [L, C2, C], F32, kind="ExternalInput")
    d_b2 = nc.dram_tensor("b2", [L, C], F32, kind="ExternalInput")
    d_Wo = nc.dram_tensor("W_out", [C, 1], F32, kind="ExternalInput")
    d_bo = nc.dram_tensor("b_out", [1, 1], F32, kind="ExternalInput")
    d_out = nc.dram_tensor("out", [G, 1], F32, kind="ExternalOutput")
    d_hdbg = nc.dram_tensor("h_dbg", [NPAD, C], F32, kind="ExternalOutput")

    # ---- inline constants ----
    eye = np.eye(128, dtype=np.float32)
    iota4_np = np.tile(np.arange(128, dtype=np.float32), (128, 4, 1))
    ones_np = np.ones((1, 128), np.float32)
    c_eye = nc.inline_tensor(eye, "c_eye")
    c_iota4b = nc.inline_tensor(iota4_np.astype(ml_dtypes.bfloat16), "c_iota4b")
    NW = PG // 128                                   # pooling windows
    iota5_np = (np.tile(np.arange(128, dtype=np.float32), (128, NW, 1))
                + (np.arange(NW, dtype=np.float32) * 128)[None, :, None])
    c_iota5 = nc.inline_tensor(iota5_np, "c_iota5")
    c_ones = nc.inline_tensor(ones_np, "c_ones")

    with tile.TileContext(nc) as tc:
        with (
            tc.tile_pool(name="dram", bufs=1, space="DRAM") as dpool,
            tc.tile_pool(name="const", bufs=1) as cp,
        ):
            # ---- internal DRAM ----
            h_locA = dpool.tile([NB, C], F32, name="h_locA")
            h_locB = dpool.tile([NB, C], F32, name="h_locB")
            h_fulls = [dpool.tile([NPAD, C], F32, addr_space=SHARED,
                                  name=f"h_full_{l}") for l in range(L)]
            e_dram = dpool.tile([E_PAD * C], F32, name="e_dram")
            pool_glob = dpool.tile([PG, C], F32, name="pool_glob")
            pool_red = dpool.tile([PG, C], F32, addr_space=SHARED, name="pool_red")

            # ---- constants to SBUF ----
            ident = cp.tile([128, 128], F32, name="ident")
            nc.sync.dma_start(ident[:, :], c_eye[:, :])
            iota4b = cp.tile([128, 4, 128], BF16, name="iota4b")
            nc.sync.dma_start(iota4b[:, :, :], c_iota4b[:, :, :])
            iota5 = cp.tile([128, NW, 128], F32, name="iota5")
            nc.sync.dma_start(iota5[:, :, :], c_iota5[:, :, :])
            pacc = cp.tile([128, NW, C], F32, name="pacc")
            nc.vector.memset(pacc[:, :, :], 0.0)
            onesr = cp.tile([1, 128], F32, name="onesr")
            nc.sync.dma_start(onesr[:, :], c_ones[:, :])

            # ---- parameter prep ----
            rhs_node = cp.tile([XD + 1, C], F32, name="rhs_node")
            nc.sync.dma_start(rhs_node[:XD, :], d_Wn[:, :])
            nc.sync.dma_start(rhs_node[XD:XD + 1, :], d_bn_[:, :])
            emit_q(nc, rhs_node[:, :])
            rhs_edge = cp.tile([ED + 1, C], F32, name="rhs_edge")
            nc.sync.dma_start(rhs_edge[:ED, :], d_We[:, :])
            nc.sync.dma_start(rhs_edge[ED:ED + 1, :], d_be[:, :])
            emit_q(nc, rhs_edge[:, :])

            def bn_rows(pref):
                g_ = cp.tile([1, C], F32, name=pref + "_g")
                b_ = cp.tile([1, C], F32, name=pref + "_b")
                m_ = cp.tile([1, C], F32, name=pref + "_m")
                sc = cp.tile([1, C], F32, name=pref + "_sc")
                bi = cp.tile([1, C], F32, name=pref + "_bi")
                nc.sync.dma_start(g_[:, :], d_bnr[pref + "_g"][:, :])
                nc.sync.dma_start(b_[:, :], d_bnr[pref + "_b"][:, :])
                nc.sync.dma_start(m_[:, :], d_bnr[pref + "_m"][:, :])
                nc.sync.dma_start(sc[:, :], d_bnr[pref + "_v"][:, :])
                nc.vector.tensor_scalar(sc[:, :], sc[:, :], BN_EPS, None, AL.add)
                nc.scalar.activation(sc[:, :], sc[:, :], ACTF.Sqrt)
                nc.vector.reciprocal(sc[:, :], sc[:, :])
                nc.vector.tensor_tensor(sc[:, :], sc[:, :], g_[:, :], op=AL.mult)
                nc.vector.tensor_tensor(bi[:, :], m_[:, :], sc[:, :], op=AL.mult)
                nc.vector.tensor_tensor(bi[:, :], b_[:, :], bi[:, :], op=AL.subtract)
                return sc, bi

            scN, biN = bn_rows("bnn")
            scE, biE = bn_rows("bne")

            def replicate4(row, nm, pool):
                ps = pool.tile([128, C], F32, name="rep_ps", tag="encp")
                nc.tensor.matmul(ps[:, :], lhsT=onesr[:, :], rhs=row[:, :],
                                 start=True, stop=True)
                out4 = cp.tile([128, 4 * C], F32, name=nm)
                for q in range(4):
                    nc.vector.tensor_copy(out4[:, q * C:(q + 1) * C], ps[:, :])
                return out4

            W1q, bias1, sc1, bi1, W2q, bias2 = [], [], [], [], [], []
            for l in range(L):
                w1 = cp.tile([C, C2], F32, name=f"W1q_{l}")
                nc.sync.dma_start(w1[:, :], d_W1[l, :, :])
                emit_q(nc, w1[:, :])
                W1q.append(w1)
                b1t = cp.tile([C2, 1], F32, name=f"bias1_{l}")
                nc.sync.dma_start(b1t[:, :], d_b1[l:l + 1, :].rearrange("a b -> b a"))
                emit_q(nc, b1t[:, :])
                nc.vector.tensor_scalar(b1t[:, :], b1t[:, :], QS, MAGIC, AL.mult, AL.add)
                bias1.append(b1t)

                g1 = cp.tile([C2, 1], F32, name=f"g1_{l}")
                bb1 = cp.tile([C2, 1], F32, name=f"bb1_{l}")
                m1 = cp.tile([C2, 1], F32, name=f"m1_{l}")
                s1 = cp.tile([C2, 1], F32, name=f"sc1_{l}")
                i1 = cp.tile([C2, 1], F32, name=f"bi1_{l}")
                nc.sync.dma_start(g1[:, :], d_bn1["bn1_g"][l:l + 1, :].rearrange("a b -> b a"))
                nc.sync.dma_start(bb1[:, :], d_bn1["bn1_b"][l:l + 1, :].rearrange("a b -> b a"))
                nc.sync.dma_start(m1[:, :], d_bn1["bn1_m"][l:l + 1, :].rearrange("a b -> b a"))
                nc.sync.dma_start(s1[:, :], d_bn1["bn1_v"][l:l + 1, :].rearrange("a b -> b a"))
                nc.vector.tensor_scalar(s1[:, :], s1[:, :], BN_EPS, None, AL.add)
                nc.scalar.activation(s1[:, :], s1[:, :], ACTF.Sqrt)
                nc.vector.reciprocal(s1[:, :], s1[:, :])
                nc.vector.tensor_tensor(s1[:, :], s1[:, :], g1[:, :], op=AL.mult)
                nc.vector.tensor_tensor(i1[:, :], m1[:, :], s1[:, :], op=AL.mult)
                nc.vector.tensor_tensor(i1[:, :], bb1[:, :], i1[:, :], op=AL.subtract)
                sc1.append(s1)
                bi1.append(i1)

                w2 = cp.tile([C2, C], F32, name=f"W2q_{l}")
                nc.sync.dma_start(w2[:, :], d_W2[l, :, :])
                emit_q(nc, w2[:, :])
                W2q.append(w2)
                b2t = cp.tile([C, 1], F32, name=f"bias2_{l}")
                nc.sync.dma_start(b2t[:, :], d_b2[l:l + 1, :].rearrange("a b -> b a"))
                emit_q(nc, b2t[:, :])
                nc.vector.tensor_scalar(b2t[:, :], b2t[:, :], QS, MAGIC, AL.mult, AL.add)
                bias2.append(b2t)

            Woq = cp.tile([C, 1], F32, name="Woq")
            nc.sync.dma_start(Woq[:, :], d_Wo[:, :])
            emit_q(nc, Woq[:, :])
            biaso = cp.tile([1, 1], F32, name="biaso")
            nc.sync.dma_start(biaso[:, :], d_bo[:, :])
            emit_q(nc, biaso[:, :])
            nc.vector.tensor_scalar(biaso[:, :], biaso[:, :], QS, MAGIC, AL.mult, AL.add)

            # ---- encoders (x / edge_attr pre-quantized on host) ----
            with (
                tc.tile_pool(name="enc", bufs=2) as enc,
                tc.tile_pool(name="encx", bufs=1) as encx,
                tc.tile_pool(name="ence", bufs=1) as ence,
                tc.tile_pool(name="encps", bufs=2, space="PSUM") as enc_ps,
            ):
                scN4 = replicate4(scN, "scN4", enc_ps)
                biN4 = replicate4(biN, "biN4", enc_ps)
                scE4 = replicate4(scE, "scE4", enc_ps)
                biE4 = replicate4(biE, "biE4", enc_ps)

                t_sb = cp.tile([1, L], F32, name="t_sb")
                nc.sync.dma_start(t_sb[:, :], d_t[:, :])
                t_ps = enc_ps.tile([128, L], F32, name="t_ps", tag="encp")
                nc.tensor.matmul(t_ps[:, :], lhsT=onesr[:, :], rhs=t_sb[:, :],
                                 start=True, stop=True)
                t_bc = cp.tile([128, L], F32, name="t_bc")
                nc.vector.tensor_copy(t_bc[:, :], t_ps[:, :])
                teps_bc = cp.tile([128, L], F32, name="teps_bc")
                nc.vector.tensor_scalar(teps_bc[:, :], t_bc[:, :], GEN_EPS, None,
                                        AL.mult)

                # node encoder
                xseg = encx.tile([XD + 1, NB], F32, name="xseg")
                nc.sync.dma_start(xseg[:, :], d_xT[:, :])
                for b in range(0, TPC, 4):
                    gs = min(4, TPC - b)
                    ep = enc_ps.tile([128, 4 * C], F32, name="encp", tag="encp")
                    for q in range(gs):
                        nc.tensor.matmul(
                            ep[:, q * C:(q + 1) * C],
                            lhsT=xseg[:, (b + q) * 128:(b + q + 1) * 128],
                            rhs=rhs_node[:, :], start=True, stop=True)
                    es = enc.tile([128, 4 * C], F32, name="encs", tag="encs")
                    nc.scalar.activation(es[:, :gs * C], ep[:, :gs * C], ACTF.Copy,
                                         bias=MAGIC, scale=QS)
                    nc.scalar.activation(es[:, :gs * C], es[:, :gs * C], ACTF.Copy,
                                         bias=QB2, scale=QI)
                    nc.vector.tensor_scalar(es[:, :gs * C], es[:, :gs * C],
                                            QMAX, QMIN, AL.min, AL.max)
                    nc.vector.tensor_tensor(es[:, :gs * C], es[:, :gs * C],
                                            scN4[:, :gs * C], op=AL.mult)
                    nc.vector.tensor_tensor(es[:, :gs * C], es[:, :gs * C],
                                            biN4[:, :gs * C], op=AL.add)
                    nc.sync.dma_start(
                        h_locA[b * 128:(b + gs) * 128, :]
                        .rearrange("(t p) c -> p t c", p=128),
                        es[:, :gs * C].rearrange("p (t c) -> p t c", c=C))

                # first AllGather (before the edge encoder so its latency
                # hides behind the edge-encoder work)
                if cfg.use_collectives:
                    nc.gpsimd.collective_compute(
                        "AllGather", AL.bypass, replica_groups=RG,
                        ins=[h_locA[:, :]], outs=[h_fulls[0][:, :]])
                else:
                    for b_ in range(cfg.ncores):
                        nc.sync.dma_start(h_fulls[0][b_ * NB:(b_ + 1) * NB, :],
                                          h_locA[:, :])

                # edge encoder: process per group so e_dram lands group-flat
                # [p][kk][c]; encoder chunk kk covers slots [p*SK+kk] for all p
                # ... wait: group-flat slot = p*SK + kk, so chunk kk is a
                # STRIDED set of flat slots.  eaT is stored in flat slot
                # order; lhsT needs 128 edges (one per PSUM partition) per
                # matmul.  We read eseg as [ED+1, SK*128] for the group and
                # matmul columns [kk*128:(kk+1)*128]?  No: flat slot order is
                # p-major, so columns p*SK+kk.  Use a strided AP instead:
                # lhsT columns for chunk kk = eseg[:, kk::SK] (stride SK).
                eav = d_eaT[:].rearrange("(r e) -> r e", r=ED + 1)
                GFoff = np.concatenate(
                    [[0], np.cumsum(np.asarray(SKg)) * 128]).astype(int)
                for Gi in range(NG):
                    SK = SKg[Gi]
                    base = int(GFoff[Gi])
                    eseg = ence.tile([ED + 1, SKMAX * 128], F32, name="eseg",
                                     tag="eseg", padded_shape=[ED + 1, SKMAX * 128])
                    nc.sync.dma_start(eseg[:, :SK * 128],
                                      eav[:, base:base + SK * 128])
                    esg = eseg[:, :SK * 128].rearrange(
                        "r (p k) -> r p k", p=128)
                    e_grp = e_dram[base * C:(base + SK * 128) * C].rearrange(
                        "(p k c) -> p k c", p=128, k=SK)
                    for b in range(0, SK, 4):
                        gs = min(4, SK - b)
                        ep = enc_ps.tile([128, 4 * C], F32, name="encp", tag="encp")
                        for q in range(gs):
                            nc.tensor.matmul(
                                ep[:, q * C:(q + 1) * C],
                                lhsT=esg[:, :, b + q],
                                rhs=rhs_edge[:, :], start=True, stop=True)
                        es = enc.tile([128, 4 * C], F32, name="encs2", tag="encs")
                        nc.scalar.activation(es[:, :gs * C], ep[:, :gs * C], ACTF.Copy,
                                             bias=MAGIC, scale=QS)
                        nc.scalar.activation(es[:, :gs * C], es[:, :gs * C], ACTF.Copy,
                                             bias=QB2, scale=QI)
                        nc.vector.tensor_scalar(es[:, :gs * C], es[:, :gs * C],
                                                QMAX, QMIN, AL.min, AL.max)
                        nc.vector.tensor_tensor(es[:, :gs * C], es[:, :gs * C],
                                                scE4[:, :gs * C], op=AL.mult)
                        nc.vector.tensor_tensor(es[:, :gs * C], es[:, :gs * C],
                                                biE4[:, :gs * C], op=AL.add)
                        nc.sync.dma_start(
                            e_grp[:, b:b + gs, :],
                            es[:, :gs * C].rearrange("p (t c) -> p t c", c=C))

            # ---- layers ----
            with (
                tc.tile_pool(name="edge", bufs=2) as epool,
                tc.tile_pool(name="node", bufs=2) as npool,
                tc.tile_pool(name="eps", bufs=2, space="PSUM") as ps_edge,
                tc.tile_pool(name="mlp1", bufs=2, space="PSUM") as ps_z1,
                tc.tile_pool(name="mlp2", bufs=1, space="PSUM") as ps_z2,
                tc.tile_pool(name="tr", bufs=2, space="PSUM") as ps_tr,
                tc.tile_pool(name="poolps", bufs=1, space="PSUM") as ps_pool,
            ):
              GFoff = np.concatenate(
                  [[0], np.cumsum(np.asarray(SKg)) * 128]).astype(int)

              for l in range(min(L, cfg.n_layers)):
                  h_in = h_locA if l % 2 == 0 else h_locB
                  h_out = h_locB if l % 2 == 0 else h_locA
                  last = l == L - 1

                  for Gi in range(NG):
                      g0 = Gi * GRP
                      gs = min(GRP, TPC - g0)
                      SK = SKg[Gi]
                      base = int(GFoff[Gi])

                      # --- load the whole group's edge data ---
                      et4 = epool.tile([128, SK, C], F32, name="et4", tag="et",
                                       padded_shape=[128, SKMAX, C])
                      nc.sync.dma_start(
                          et4[:, :, :],
                          e_dram[base * C:(base + SK * 128) * C]
                          .rearrange("(p k c) -> p k c", p=128, k=SK))
                      idxt4 = epool.tile([128, SK], I32, name="idxt4", tag="idxt",
                                         padded_shape=[128, SKMAX])
                      nc.sync.dma_start(
                          idxt4[:, :],
                          d_gidx[base:base + SK * 128]
                          .rearrange("(p k) -> p k", p=128))
                      dlt4 = epool.tile([128, SK], BF16, name="dlt4", tag="dlt",
                                        padded_shape=[128, SKMAX])
                      nc.sync.dma_start(
                          dlt4[:, :],
                          d_dloc[base:base + SK * 128]
                          .rearrange("(p k) -> p k", p=128))
                      hog = npool.tile([128, GRP, C], F32, name="hog", tag="hog")
                      nc.sync.dma_start(
                          hog[:, :gs, :],
                          h_in[g0 * 128:(g0 + gs) * 128, :]
                          .rearrange("(t p) c -> p t c", p=128))

                      # --- gather h[src] (+e), one [128,1]-indexed indirect
                      # DMA per chunk-column (HW consumes ONE index per
                      # partition per instruction; wider offset APs silently
                      # degrade into a consecutive-row block gather) ---
                      if cfg.gather_acc:
                          for kk in range(SK):
                              nc.gpsimd.indirect_dma_start(
                                  out=et4[:, kk, :], out_offset=None,
                                  in_=h_fulls[l][:, :],
                                  in_offset=IndirectOffsetOnAxis(
                                      ap=idxt4[:, kk:kk + 1], axis=0),
                                  compute_op=AL.add)
                      else:
                          hsg4 = epool.tile([128, SK, C], F32, name="hsg4",
                                            tag="hsg",
                                            padded_shape=[128, SKMAX, C])
                          for kk in range(SK):
                              nc.gpsimd.indirect_dma_start(
                                  out=hsg4[:, kk, :], out_offset=None,
                                  in_=h_fulls[l][:, :],
                                  in_offset=IndirectOffsetOnAxis(
                                      ap=idxt4[:, kk:kk + 1], axis=0))
                          nc.vector.tensor_tensor(et4[:, :, :], et4[:, :, :],
                                                  hsg4[:, :, :], op=AL.add)

                      # r = relu(h_src + e); ex = exp(t*r + t*eps) in bf16;
                      # num-side = ex * r (the +eps on m is folded into h2)
                      nc.scalar.activation(et4[:, :, :], et4[:, :, :], ACTF.Relu)
                      exm4 = epool.tile([128, SK, C2], BF16, name="exm4",
                                        tag="exm",
                                        padded_shape=[128, SKMAX, C2])
                      nc.scalar.activation(exm4[:, :, C:C2], et4[:, :, :],
                                           ACTF.Exp, bias=teps_bc[:, l:l + 1],
                                           scale=t_bc[:, l:l + 1])
                      nc.vector.tensor_copy(exm4[:, :, 0:C], et4[:, :, :])
                      nc.vector.tensor_tensor(exm4[:, :, 0:C], exm4[:, :, 0:C],
                                              exm4[:, :, C:C2], op=AL.mult)

                      # --- per tile: one-hot scatter + h2 + transpose + MLP ---
                      h2qT = npool.tile([C, 512], F32, name="h2qT", tag="h2qT")
                      off = 0
                      for t in range(gs):
                          g = g0 + t
                          K = Kg[g]
                          eps_t = ps_edge.tile([128, C2], F32, name="eps_t",
                                               tag="eps_t")
                          for j0 in range(0, K, 4):
                              jj = min(4, K - j0)
                              oh4 = epool.tile([128, 4, 128], BF16, name="oh4",
                                               tag="oh4")
                              nc.vector.tensor_tensor(
                                  oh4[:, :jj, :],
                                  dlt4[:, off + j0:off + j0 + jj]
                                  .to_broadcast([128, jj, 128]),
                                  iota4b[:, :jj, :], op=AL.is_equal)
                              for q in range(jj):
                                  j = j0 + q
                                  nc.tensor.matmul(
                                      eps_t[:, :], lhsT=oh4[:, q, :],
                                      rhs=exm4[:, off + j, :],
                                      start=(j == 0), stop=(j == K - 1))

                          # agg = num/max(den,1e-16) + eps;  h2 = h_own + agg
                          dinv = npool.tile([128, C], F32, name="dinv", tag="dinv")
                          nc.vector.tensor_scalar(dinv[:, :], eps_t[:, C:C2],
                                                  1e-16, None, AL.max)
                          nc.vector.reciprocal(dinv[:, :], dinv[:, :])
                          h2 = npool.tile([128, C], F32, name="h2", tag="h2")
                          nc.vector.tensor_tensor(h2[:, :], eps_t[:, 0:C],
                                                  dinv[:, :], op=AL.mult)
                          nc.vector.tensor_scalar(h2[:, :], h2[:, :], GEN_EPS,
                                                  None, AL.add)
                          nc.vector.tensor_tensor(h2[:, :], h2[:, :],
                                                  hog[:, t, :], op=AL.add)
                          emit_q(nc, h2[:, :])
                          trp = ps_tr.tile([C, 128], F32, name="trp", tag="tr")
                          nc.tensor.transpose(trp[:, :], h2[:, :],
                                              identity=ident[:, :])
                          nc.vector.tensor_copy(h2qT[:, t * 128:(t + 1) * 128],
                                                trp[:, :])
                          off += K

                      # --- MLP on the group (up to 4 node tiles) ---
                      w = gs * 128
                      z1p = ps_z1.tile([C2, 512], F32, name="z1p", tag="z1p")
                      nc.tensor.matmul(z1p[:, :w], lhsT=W1q[l][:, :],
                                       rhs=h2qT[:, :w], start=True, stop=True)
                      z1s = npool.tile([C2, 512], F32, name="z1s", tag="z1s")
                      nc.scalar.activation(z1s[:, :w], z1p[:, :w], ACTF.Identity,
                                           bias=bias1[l][:, :], scale=QS)
                      nc.scalar.activation(z1s[:, :w], z1s[:, :w], ACTF.Copy,
                                           bias=QB2, scale=QI)
                      nc.vector.tensor_scalar(z1s[:, :w], z1s[:, :w], QMAX, QMIN,
                                              AL.min, AL.max)
                      nc.scalar.activation(z1s[:, :w], z1s[:, :w], ACTF.Relu,
                                           bias=bi1[l][:, :], scale=sc1[l][:, :])
                      nc.scalar.activation(z1s[:, :w], z1s[:, :w], ACTF.Copy,
                                           bias=MAGIC, scale=QS)
                      nc.scalar.activation(z1s[:, :w], z1s[:, :w], ACTF.Copy,
                                           bias=QB2, scale=QI)
                      nc.vector.tensor_scalar(z1s[:, :w], z1s[:, :w], QMAX, QMIN,
                                              AL.min, AL.max)
                      z2p = ps_z2.tile([C, 512], F32, name="z2p", tag="z2p")
                      nc.tensor.matmul(z2p[:, :w], lhsT=W2q[l][:, :],
                                       rhs=z1s[:, :w], start=True, stop=True)
                      z2s = npool.tile([C, 512], F32, name="z2s", tag="z2s")
                      nc.scalar.activation(z2s[:, :w], z2p[:, :w], ACTF.Identity,
                                           bias=bias2[l][:, :], scale=QS)
                      nc.scalar.activation(z2s[:, :w], z2s[:, :w], ACTF.Copy,
                                           bias=QB2, scale=QI)
                      nc.vector.tensor_scalar(z2s[:, :w], z2s[:, :w], QMAX, QMIN,
                                              AL.min, AL.max)
                      hnext = npool.tile([128, GRP, C], F32, name="hnext",
                                         tag="hnext")
                      for q in range(gs):
                          trq = ps_tr.tile([128, C], F32, name="trq", tag="tr")
                          nc.tensor.transpose(trq[:, :],
                                              z2s[:, q * 128:(q + 1) * 128],
                                              identity=ident[0:C, 0:C])
                          nc.vector.tensor_tensor(hnext[:, q, :], trq[:, :],
                                                  hog[:, q, :], op=AL.add)
                          if last:
                              blt = npool.tile([128, 1], F32, name="blt", tag="blt")
                              nc.sync.dma_start(
                                  blt[:, :],
                                  d_bloc[(g0 + q) * 128:(g0 + q + 1) * 128]
                                  .rearrange("(p one) -> p one", one=1))
                              ohp = npool.tile([128, NW, 128], F32, name="ohp",
                                               tag="ohp")
                              nc.vector.tensor_tensor(
                                  ohp[:, :, :],
                                  blt[:, :].to_broadcast([128, NW, 128]),
                                  iota5[:, :, :], op=AL.is_equal)
                              for wi in range(NW):
                                  pps = ps_pool.tile([128, C], F32, name="pps",
                                                     tag="pps")
                                  nc.tensor.matmul(
                                      pps[:, :], lhsT=ohp[:, wi, :],
                                      rhs=hnext[:, q, :],
                                      start=True, stop=True)
                                  nc.vector.tensor_tensor(
                                      pacc[:, wi, :], pacc[:, wi, :], pps[:, :],
                                      op=AL.add)
                      if not last:
                          nc.sync.dma_start(
                              h_out[g0 * 128:(g0 + gs) * 128, :]
                              .rearrange("(t p) c -> p t c", p=128),
                              hnext[:, :gs, :])

                  if not last:
                      if cfg.use_collectives:
                          nc.gpsimd.collective_compute(
                              "AllGather", AL.bypass, replica_groups=RG,
                              ins=[h_out[:, :]], outs=[h_fulls[l + 1][:, :]])
                      else:
                          for b_ in range(cfg.ncores):
                              nc.sync.dma_start(
                                  h_fulls[l + 1][b_ * NB:(b_ + 1) * NB, :],
                                  h_out[:, :])

              if cfg.n_layers < L:
                  nl = cfg.n_layers
                  hf = h_fulls[min(nl, L - 1)]
                  for b_ in range(NPAD // 128):
                      dbg_t = npool.tile([128, C], F32, name="dbg_t", tag="dbg_t")
                      nc.sync.dma_start(dbg_t[:, :],
                                        hf[b_ * 128:(b_ + 1) * 128, :])
                      nc.sync.dma_start(d_hdbg[b_ * 128:(b_ + 1) * 128, :],
                                        dbg_t[:, :])
                  return nc

              # ---- pooling: write window partials, AllReduce, output head ----
              nc.sync.dma_start(
                  pool_glob[:, :].rearrange("(w p) c -> p w c", p=128),
                  pacc[:, :, :])
              if cfg.use_collectives:
                  nc.gpsimd.collective_compute(
                      "AllReduce", AL.add, replica_groups=RG,
                      ins=[pool_glob[:, :]], outs=[pool_red[:, :]])
              else:
                  nc.sync.dma_start(pool_red[:, :], pool_glob[:, :])

              n_out_tiles = (G + 127) // 128
              for i in range(n_out_tiles):
                  w = min(128, G - i * 128)
                  pt = npool.tile([128, C], F32, name="pt", tag="pt")
                  nc.sync.dma_start(pt[:w, :], pool_red[i * 128:i * 128 + w, :])
                  civ = npool.tile([128, 1], F32, name="civ", tag="civ")
                  nc.sync.dma_start(civ[:w, :],
                                    d_cntinv[i * 128:i * 128 + w].rearrange("(p one) -> p one", one=1))
                  nc.vector.tensor_scalar(pt[:w, :], pt[:w, :], civ[:w, :], None, AL.mult)
                  emit_q(nc, pt[:w, :])
                  trh = ps_tr.tile([C, 128], F32, name="trh", tag="tr")
                  nc.tensor.transpose(trh[:, :w], pt[:w, :], identity=ident[:w, :w])
                  hts = npool.tile([C, 128], F32, name="hts", tag="hts")
                  nc.vector.tensor_copy(hts[:, :w], trh[:, :w])
                  op_ = ps_z2.tile([1, 128], F32, name="op_", tag="z2p")
                  nc.tensor.matmul(op_[:, :w], lhsT=Woq[:, :], rhs=hts[:, :w],
                                   start=True, stop=True)
                  osb = npool.tile([1, 128], F32, name="osb", tag="osb")
                  nc.scalar.activation(osb[:, :w], op_[:, :w], ACTF.Identity,
                                       bias=biaso[:, :], scale=QS)
                  nc.scalar.activation(osb[:, :w], osb[:, :w], ACTF.Copy,
                                       bias=QB2, scale=QI)
                  nc.vector.tensor_scalar(osb[:, :w], osb[:, :w], QMAX, QMIN,
                                          AL.min, AL.max)
                  nc.scalar.activation(osb[:, :w], osb[:, :w], ACTF.Sigmoid)
                  nc.scalar.activation(osb[:, :w], osb[:, :w], ACTF.Copy,
                                       bias=MAGIC, scale=QS)
                  nc.scalar.activation(osb[:, :w], osb[:, :w], ACTF.Copy,
                                       bias=QB2, scale=QI)
                  nc.sync.dma_start(
                      d_out[i * 128:i * 128 + w, :].rearrange("w one -> one w"),
                      osb[:, :w])

    return nc


# ----------------------------------------------------------------------------
# Entry point.
# ----------------------------------------------------------------------------

def run(inputs, cfg, **run_kwargs):
    global LAST_RESULTS
    in_maps = preprocess(inputs, cfg)
    nc = build(cfg)
    if not nc.is_finalized():
        nc.finalize()
    res = run_bass_kernel_spmd(nc, in_maps, core_ids=list(range(cfg.ncores)),
                               **run_kwargs)
    LAST_RESULTS = res
    return res.results[0]["out"].reshape(cfg.G, 1).astype(np.float32)


def kernel(**inputs) -> np.ndarray:
    cfg = Cfg(N=100000, E=3200000, G=512, XD=8, ED=4, C=32, L=4)
    return run(inputs, cfg)


# revision 25
# speedup vs baseline: 1.2546x; 1.0611x over previous
"""Trainium2 Bass kernel: nn_BV_Model (GENConv GNN, softmax aggregation, 4 layers).

Strategy (8 NeuronCores, SPMD):
  - Nodes are partitioned into 8 contiguous blocks (12544/core, padded).
  - Edges are sorted by destination node and bucketed per destination
    node-tile (128 nodes); each core owns the edges whose dst falls in its
    block.  Tiles are processed in groups of 4; within a group the edge
    slots are laid out "group-flat" [p][kk][c] (p = SBUF partition,
    kk = chunk slot within the group) so each group's edge features /
    src indices / dst offsets load as one large DMA and h[src] is fetched
    with ONE batched indirect DMA (~13k descriptors) per group.
  - Per layer: m = relu(h[src]+e), ex = exp(t*m) (bf16), and the segment
    softmax numerator/denominator reduce edges->nodes with one-hot(dst)
    matmuls (bf16) accumulated in PSUM.  No segment-max: s_max ~ 65,
    exp fits fp32/bf16 range (verified offline).  Node MLP runs on the
    tensor engine in transposed layout.  h is AllGathered per layer.
  - Global mean pool via one-hot(graph) matmuls, AllReduce, output head.

Fake-quantization q(x) = clip(rne(x*1024), -32768, 32767)/1024 is computed
exactly with the round-to-nearest-even "magic number" trick (+1.5*2^23).
x/edge_attr are pre-quantized on the host (same RNE semantics).
"""

import os
os.environ.setdefault("MYCRO_LOCAL_CACHE", "1")

import math
import numpy as np
import ml_dtypes

import concourse.bacc as bacc
import concourse.tile as tile
import concourse.bass as bass
from concourse import mybir
from concourse.bass import IndirectOffsetOnAxis
from concourse.bass_utils import run_bass_kernel_spmd

F32 = mybir.dt.float32
BF16 = mybir.dt.bfloat16
I32 = mybir.dt.int32
ACTF = mybir.ActivationFunctionType
AL = mybir.AluOpType

MAGIC = 12582912.0           # 1.5*2^23 : fp32 RNE rounding magic
QS = 1024.0                  # 2^10
QI = 1.0 / 1024.0
QB2 = -12288.0               # -MAGIC * 2^-10
QMAX = 32767.0 / 1024.0
QMIN = -32.0
GEN_EPS = 1e-7
BN_EPS = 1e-5
NCORES = 8
GRP = 4                      # node tiles per group (shared w/ MLP batching)

LAST_RESULTS = None          # BassKernelResults of the most recent run (for test.py)


class Cfg:
    def __init__(self, N, E, G, XD=8, ED=4, C=32, L=4, ncores=NCORES,
                 use_collectives=True, use_shared=True,
                 gather_acc=False, gather_max_cols=64, n_layers=None):
        self.N, self.E, self.G = N, E, G
        self.XD, self.ED, self.C, self.L = XD, ED, C, L
        self.ncores = ncores
        self.use_collectives = use_collectives and ncores > 1
        self.use_shared = use_shared
        self.gather_acc = gather_acc
        self.gather_max_cols = gather_max_cols
        self.n_layers = L if n_layers is None else n_layers
        self.TPC = (N + ncores * 128 - 1) // (ncores * 128)    # node tiles per core
        self.NB = self.TPC * 128                               # nodes per core (padded)
        self.NPAD = self.NB * ncores
        self.PG = ((G + 128) + 127) // 128 * 128               # pooled scatter rows
        self.SKg = None                                        # chunks per group [NG]
        self.E_PAD = None                                      # padded edges per core
        self.tfrag = None                                      # per group tile chunk frags
        self.gspans = None                                     # per group gather spans
        self.IDXW = None
        self.NR = None
        self.RMAX = 32768


def qnp(a):
    """Host-side ap_fixed<16,6> fake quantization (RNE, matches HW magic)."""
    y = np.round(a.astype(np.float64) * QS) * QI
    return np.clip(y, QMIN, QMAX).astype(np.float32)


# ----------------------------------------------------------------------------
# Host-side preprocessing: sort/bucket edges, build per-core input arrays.
# ----------------------------------------------------------------------------

def preprocess(inputs, cfg):
    x = qnp(np.asarray(inputs["x"], np.float32))
    ea = qnp(np.asarray(inputs["edge_attr"], np.float32))
    ei = np.asarray(inputs["edge_index"]).astype(np.int64)
    batch = np.asarray(inputs["batch"]).astype(np.int64)
    N, E, G = cfg.N, cfg.E, cfg.G
    XD, ED, C, L = cfg.XD, cfg.ED, cfg.C, cfg.L
    TPC, NB = cfg.TPC, cfg.NB
    NC_ = cfg.ncores

    src, dst = ei[0], ei[1]
    # sort by (dst tile, src): tile bucketing unchanged, but edges within a
    # tile then gather ascending clustered h rows (HBM locality)
    order = np.argsort((dst // 128).astype(np.int64) * (N + 1) + src,
                       kind="stable")
    src_s = src[order]
    dst_s = dst[order]
    ea_s = ea[order]

    ntiles = NC_ * TPC
    bnd = np.searchsorted(dst_s, np.arange(ntiles + 1) * 128)
    NG = (TPC + GRP - 1) // GRP
    RMAX = 32768
    NR = (cfg.NPAD + RMAX - 1) // RMAX           # src ranges (int16 index cap)
    GCH = 8                                      # chunks per dma_gather (1024 idx)

    # per (core, tile, range) edge counts (ranges contiguous: src-sorted)
    cnt_r = np.zeros((NC_, TPC, NR), np.int64)
    roff = np.zeros((NC_, TPC, NR), np.int64)
    for c in range(NC_):
        for g in range(TPC):
            tl = c * TPC + g
            seg = src_s[bnd[tl]:bnd[tl + 1]]
            rb = np.searchsorted(seg, np.arange(1, NR + 1) * RMAX)
            prev = 0
            for r in range(NR):
                cnt_r[c, g, r] = rb[r] - prev
                roff[c, g, r] = prev
                prev = rb[r]
    K_gr = ((cnt_r + 127) // 128).max(axis=0)    # [TPC, NR] shared chunks
    for g in range(TPC):
        if K_gr[g].sum() == 0:
            K_gr[g, 0] = 1                       # >=1 chunk per tile
    SKg = np.array([int(K_gr[g0:g0 + GRP].sum())
                    for g0 in range(0, TPC, GRP)], np.int64)  # [NG]
    E_PAD = int(K_gr.sum()) * 128
    GFoff = np.concatenate([[0], np.cumsum(SKg) * 128])

    # group slot order (range r, tile t, chunk j); flat = base + p*SK + kk
    rspan = []                                   # per group: [(kk0, kk1)]*NR
    tfrag = []                                   # per group: t -> [(kk0, K)]
    for Gi in range(NG):
        g0 = Gi * GRP
        gs = min(GRP, TPC - g0)
        rs, tf, nck = [], [[] for _ in range(gs)], 0
        for r in range(NR):
            kk0 = nck
            for t in range(gs):
                K = int(K_gr[g0 + t, r])
                if K:
                    tf[t].append((nck, K))
                nck += K
            rs.append((kk0, nck))
        rspan.append(rs)
        tfrag.append(tf)

    gspans = []                                  # per group: [(kk0, nj, col0, r)]
    col = 0
    for Gi in range(NG):
        spans = []
        for r in range(NR):
            kk0, kk1 = rspan[Gi][r]
            a = kk0
            while a < kk1:
                nj = min(GCH, kk1 - a)
                spans.append((a, nj, col, r))
                col += nj * 8
                a += nj
        gspans.append(spans)
    IDXW = col                                   # int16 idx cols (16-wrapped)

    idx16_a = np.zeros((NC_, 128, IDXW), np.int16)
    dloc_a = np.full((NC_, E_PAD), -1.0, ml_dtypes.bfloat16)
    eaT_a = np.zeros((NC_, ED + 1, E_PAD), np.float32)
    eaT_a[:, ED, :] = 1.0
    for c in range(NC_):
        for Gi in range(NG):
            g0 = Gi * GRP
            gs = min(GRP, TPC - g0)
            SK = int(SKg[Gi])
            base = int(GFoff[Gi])
            loc_idx = np.zeros((128, SK), np.int64)
            kkc = 0
            for r in range(NR):
                for t in range(gs):
                    g = g0 + t
                    tl = c * TPC + g
                    K = int(K_gr[g, r])
                    if K == 0:
                        continue
                    m = int(cnt_r[c, g, r])
                    if m > 0:
                        i_ar = np.arange(m)
                        rows = bnd[tl] + int(roff[c, g, r]) + i_ar
                        pp = i_ar % 128
                        kk = kkc + i_ar // 128
                        flat = base + pp * SK + kk
                        dloc_a[c, flat] = (dst_s[rows] - tl * 128
                                           ).astype(np.float32)
                        # advanced (flat) axis comes FIRST -> assign [m, ED]
                        eaT_a[c, :ED, flat] = ea_s[rows]
                        loc_idx[pp, kk] = src_s[rows] - r * RMAX
                    kkc += K
            for (a, nj, c0, r) in gspans[Gi]:
                ni = nj * 128
                i_ar = np.arange(ni)
                vals = loc_idx[i_ar % 128, a + i_ar // 128]
                blk = np.zeros((16, nj * 8), np.int16)
                blk[i_ar % 16, i_ar // 16] = vals.astype(np.int16)
                idx16_a[c, :, c0:c0 + nj * 8] = np.tile(blk, (8, 1))

    xT_a = np.zeros((NC_, XD + 1, NB), np.float32)
    xT_a[:, XD, :] = 1.0
    bloc_a = np.full((NC_, NB), -1.0, np.float32)
    first_g = np.zeros(NC_, np.int64)
    for c in range(NC_):
        lo, hi = c * NB, min((c + 1) * NB, N)
        xT_a[c, :XD, : hi - lo] = x[lo:hi].T
        first_g[c] = batch[lo]
        assert batch[hi - 1] - batch[lo] < 128, "graph window exceeds 128"
        bloc_a[c, : hi - lo] = batch[lo:hi].astype(np.float32)

    prow_a = (first_g[:, None] + np.arange(128)[None, :]).astype(np.int32)
    assert prow_a.max() < cfg.PG
    cnt_g = np.bincount(batch, minlength=G).astype(np.float32)
    cnt_inv = np.zeros(cfg.PG, np.float32)
    cnt_inv[:G] = np.float32(1.0) / np.maximum(cnt_g, np.float32(1.0))

    cfg.SKg = [int(k) for k in SKg]
    cfg.E_PAD = E_PAD
    cfg.tfrag = tfrag
    cfg.gspans = gspans
    cfg.IDXW = IDXW
    cfg.NR = NR
    cfg.RMAX = RMAX

    def f32(a):
        return np.ascontiguousarray(np.asarray(a, np.float32))

    shared = dict(
        W_node=f32(inputs["W_node"]), b_node=f32(inputs["b_node"]).reshape(1, C),
        W_edge=f32(inputs["W_edge"]), b_edge=f32(inputs["b_edge"]).reshape(1, C),
        bnn_g=f32(inputs["bnn_g"]).reshape(1, C), bnn_b=f32(inputs["bnn_b"]).reshape(1, C),
        bnn_m=f32(inputs["bnn_m"]).reshape(1, C), bnn_v=f32(inputs["bnn_v"]).reshape(1, C),
        bne_g=f32(inputs["bne_g"]).reshape(1, C), bne_b=f32(inputs["bne_b"]).reshape(1, C),
        bne_m=f32(inputs["bne_m"]).reshape(1, C), bne_v=f32(inputs["bne_v"]).reshape(1, C),
        t=f32(inputs["t"]).reshape(1, L),
        W1=f32(inputs["W1"]), b1=f32(inputs["b1"]),
        bn1_g=f32(inputs["bn1_g"]), bn1_b=f32(inputs["bn1_b"]),
        bn1_m=f32(inputs["bn1_m"]), bn1_v=f32(inputs["bn1_v"]),
        W2=f32(inputs["W2"]), b2=f32(inputs["b2"]),
        W_out=f32(inputs["W_out"]), b_out=f32(inputs["b_out"]).reshape(1, 1),
        cnt_inv=cnt_inv,
    )
    in_maps = []
    for c in range(NC_):
        im = dict(shared)
        im.update(
            xT=xT_a[c], eaT=eaT_a[c].reshape(-1), gidx16=idx16_a[c],
            dloc=dloc_a[c], bloc=bloc_a[c], prow=prow_a[c],
        )
        in_maps.append(im)
    return in_maps


# ----------------------------------------------------------------------------
# Device program.
# ----------------------------------------------------------------------------

def emit_q(nc, ap, pre_bias_ap=None, clip=True):
    """In-place fake quantization of `ap` (fp32): q(x) (+fused bias if given).

    If pre_bias_ap is given it must hold (1024*bias_q + MAGIC) per partition and
    the op computes q(x + bias_q)."""
    if pre_bias_ap is None:
        nc.scalar.activation(ap, ap, ACTF.Copy, bias=MAGIC, scale=QS)
    else:
        nc.scalar.activation(ap, ap, ACTF.Identity, bias=pre_bias_ap, scale=QS)
    nc.scalar.activation(ap, ap, ACTF.Copy, bias=QB2, scale=QI)
    if clip:
        nc.vector.tensor_scalar(ap, ap, QMAX, QMIN, AL.min, AL.max)


def build(cfg):
    C, L, TPC, NB = cfg.C, cfg.L, cfg.TPC, cfg.NB
    XD, ED, G, PG = cfg.XD, cfg.ED, cfg.G, cfg.PG
    NPAD, E_PAD, SKg = cfg.NPAD, cfg.E_PAD, cfg.SKg
    HW = 2 * C                      # h row width in DRAM (256B for dma_gather)
    C2 = 2 * C
    NG = (TPC + GRP - 1) // GRP
    SKMAX = max(SKg)
    RG = [list(range(cfg.ncores))]
    SHARED = "Shared" if (cfg.use_shared and cfg.use_collectives) else "Local"

    nc = bacc.Bacc("TRN2", target_bir_lowering=False, debug=False,
                   enable_asserts=False, num_devices=cfg.ncores)

    # ---- kernel I/O ----
    d_xT = nc.dram_tensor("xT", [XD + 1, NB], F32, kind="ExternalInput")
    d_eaT = nc.dram_tensor("eaT", [(ED + 1) * E_PAD], F32, kind="ExternalInput")
    I16 = mybir.dt.int16
    d_gidx16 = nc.dram_tensor("gidx16", [128, cfg.IDXW], I16,
                              kind="ExternalInput")
    d_dloc = nc.dram_tensor("dloc", [E_PAD], BF16, kind="ExternalInput")
    d_bloc = nc.dram_tensor("bloc", [NB], F32, kind="ExternalInput")
    d_prow = nc.dram_tensor("prow", [128], I32, kind="ExternalInput")
    d_cntinv = nc.dram_tensor("cnt_inv", [PG], F32, kind="ExternalInput")
    d_Wn = nc.dram_tensor("W_node", [XD, C], F32, kind="ExternalInput")
    d_bn_ = nc.dram_tensor("b_node", [1, C], F32, kind="ExternalInput")
    d_We = nc.dram_tensor("W_edge", [ED, C], F32, kind="ExternalInput")
    d_be = nc.dram_tensor("b_edge", [1, C], F32, kind="ExternalInput")
    d_bnr = {k: nc.dram_tensor(k, [1, C], F32, kind="ExternalInput")
             for k in ["bnn_g", "bnn_b", "bnn_m", "bnn_v",
                       "bne_g", "bne_b", "bne_m", "bne_v"]}
    d_t = nc.dram_tensor("t", [1, L], F32, kind="ExternalInput")
    d_W1 = nc.dram_tensor("W1", [L, C, C2], F32, kind="ExternalInput")
    d_b1 = nc.dram_tensor("b1", [L, C2], F32, kind="ExternalInput")
    d_bn1 = {k: nc.dram_tensor(k, [L, C2], F32, kind="ExternalInput")
             for k in ["bn1_g", "bn1_b", "bn1_m", "bn1_v"]}
    d_W2 = nc.dram_tensor("W2", [L, C2, C], F32, kind="ExternalInput")
    d_b2 = nc.dram_tensor("b2", [L, C], F32, kind="ExternalInput")
    d_Wo = nc.dram_tensor("W_out", [C, 1], F32, kind="ExternalInput")
    d_bo = nc.dram_tensor("b_out", [1, 1], F32, kind="ExternalInput")
    d_out = nc.dram_tensor("out", [G, 1], F32, kind="ExternalOutput")
    d_hdbg = nc.dram_tensor("h_dbg", [NPAD, C], F32, kind="ExternalOutput")

    # ---- inline constants ----
    eye = np.eye(128, dtype=np.float32)
    iota4_np = np.tile(np.arange(128, dtype=np.float32), (128, 4, 1))
    ones_np = np.ones((1, 128), np.float32)
    c_eye = nc.inline_tensor(eye, "c_eye")
    c_iota4b = nc.inline_tensor(iota4_np.astype(ml_dtypes.bfloat16), "c_iota4b")
    NW = PG // 128                                   # pooling windows
    iota5_np = (np.tile(np.arange(128, dtype=np.float32), (128, NW, 1))
                + (np.arange(NW, dtype=np.float32) * 128)[None, :, None])
    c_iota5 = nc.inline_tensor(iota5_np, "c_iota5")
    c_ones = nc.inline_tensor(ones_np, "c_ones")

    with tile.TileContext(nc) as tc:
        with (
            tc.tile_pool(name="dram", bufs=1, space="DRAM") as dpool,
            tc.tile_pool(name="const", bufs=1) as cp,
        ):
            # ---- internal DRAM ----
            # h rows are 64 fp32 wide (256B) so dma_gather's elem-size
            # constraint holds; only cols 0:C are meaningful
            h_locA = dpool.tile([NB, HW], F32, name="h_locA")
            h_locB = dpool.tile([NB, HW], F32, name="h_locB")
            h_fulls = [dpool.tile([NPAD, HW], F32, addr_space=SHARED,
                                  name=f"h_full_{l}") for l in range(L)]
            e_dram = dpool.tile([E_PAD * C], F32, name="e_dram")
            pool_glob = dpool.tile([PG, C], F32, name="pool_glob")
            pool_red = dpool.tile([PG, C], F32, addr_space=SHARED, name="pool_red")

            # ---- constants to SBUF ----
            ident = cp.tile([128, 128], F32, name="ident")
            nc.sync.dma_start(ident[:, :], c_eye[:, :])
            iota4b = cp.tile([128, 4, 128], BF16, name="iota4b")
            nc.sync.dma_start(iota4b[:, :, :], c_iota4b[:, :, :])
            iota5 = cp.tile([128, NW, 128], F32, name="iota5")
            nc.sync.dma_start(iota5[:, :, :], c_iota5[:, :, :])
            pacc = cp.tile([128, NW, C], F32, name="pacc")
            nc.vector.memset(pacc[:, :, :], 0.0)
            onesr = cp.tile([1, 128], F32, name="onesr")
            nc.sync.dma_start(onesr[:, :], c_ones[:, :])

            # ---- parameter prep ----
            rhs_node = cp.tile([XD + 1, C], F32, name="rhs_node")
            nc.sync.dma_start(rhs_node[:XD, :], d_Wn[:, :])
            nc.sync.dma_start(rhs_node[XD:XD + 1, :], d_bn_[:, :])
            emit_q(nc, rhs_node[:, :])
            rhs_edge = cp.tile([ED + 1, C], F32, name="rhs_edge")
            nc.sync.dma_start(rhs_edge[:ED, :], d_We[:, :])
            nc.sync.dma_start(rhs_edge[ED:ED + 1, :], d_be[:, :])
            emit_q(nc, rhs_edge[:, :])

            def bn_rows(pref):
                g_ = cp.tile([1, C], F32, name=pref + "_g")
                b_ = cp.tile([1, C], F32, name=pref + "_b")
                m_ = cp.tile([1, C], F32, name=pref + "_m")
                sc = cp.tile([1, C], F32, name=pref + "_sc")
                bi = cp.tile([1, C], F32, name=pref + "_bi")
                nc.sync.dma_start(g_[:, :], d_bnr[pref + "_g"][:, :])
                nc.sync.dma_start(b_[:, :], d_bnr[pref + "_b"][:, :])
                nc.sync.dma_start(m_[:, :], d_bnr[pref + "_m"][:, :])
                nc.sync.dma_start(sc[:, :], d_bnr[pref + "_v"][:, :])
                nc.vector.tensor_scalar(sc[:, :], sc[:, :], BN_EPS, None, AL.add)
                nc.scalar.activation(sc[:, :], sc[:, :], ACTF.Sqrt)
                nc.vector.reciprocal(sc[:, :], sc[:, :])
                nc.vector.tensor_tensor(sc[:, :], sc[:, :], g_[:, :], op=AL.mult)
                nc.vector.tensor_tensor(bi[:, :], m_[:, :], sc[:, :], op=AL.mult)
                nc.vector.tensor_tensor(bi[:, :], b_[:, :], bi[:, :], op=AL.subtract)
                return sc, bi

            scN, biN = bn_rows("bnn")
            scE, biE = bn_rows("bne")

            def replicate4(row, nm, pool):
                ps = pool.tile([128, C], F32, name="rep_ps", tag="encp")
                nc.tensor.matmul(ps[:, :], lhsT=onesr[:, :], rhs=row[:, :],
                                 start=True, stop=True)
                out4 = cp.tile([128, 4 * C], F32, name=nm)
                for q in range(4):
                    nc.vector.tensor_copy(out4[:, q * C:(q + 1) * C], ps[:, :])
                return out4

            W1q, bias1, sc1, bi1, W2q, bias2 = [], [], [], [], [], []
            for l in range(L):
                w1 = cp.tile([C, C2], F32, name=f"W1q_{l}")
                nc.sync.dma_start(w1[:, :], d_W1[l, :, :])
                emit_q(nc, w1[:, :])
                W1q.append(w1)
                b1t = cp.tile([C2, 1], F32, name=f"bias1_{l}")
                nc.sync.dma_start(b1t[:, :], d_b1[l:l + 1, :].rearrange("a b -> b a"))
                emit_q(nc, b1t[:, :])
                nc.vector.tensor_scalar(b1t[:, :], b1t[:, :], QS, MAGIC, AL.mult, AL.add)
                bias1.append(b1t)

                g1 = cp.tile([C2, 1], F32, name=f"g1_{l}")
                bb1 = cp.tile([C2, 1], F32, name=f"bb1_{l}")
                m1 = cp.tile([C2, 1], F32, name=f"m1_{l}")
                s1 = cp.tile([C2, 1], F32, name=f"sc1_{l}")
                i1 = cp.tile([C2, 1], F32, name=f"bi1_{l}")
                nc.sync.dma_start(g1[:, :], d_bn1["bn1_g"][l:l + 1, :].rearrange("a b -> b a"))
                nc.sync.dma_start(bb1[:, :], d_bn1["bn1_b"][l:l + 1, :].rearrange("a b -> b a"))
                nc.sync.dma_start(m1[:, :], d_bn1["bn1_m"][l:l + 1, :].rearrange("a b -> b a"))
                nc.sync.dma_start(s1[:, :], d_bn1["bn1_v"][l:l + 1, :].rearrange("a b -> b a"))
                nc.vector.tensor_scalar(s1[:, :], s1[:, :], BN_EPS, None, AL.add)
                nc.scalar.activation(s1[:, :], s1[:, :], ACTF.Sqrt)
                nc.vector.reciprocal(s1[:, :], s1[:, :])
                nc.vector.tensor_tensor(s1[:, :], s1[:, :], g1[:, :], op=AL.mult)
                nc.vector.tensor_tensor(i1[:, :], m1[:, :], s1[:, :], op=AL.mult)
                nc.vector.tensor_tensor(i1[:, :], bb1[:, :], i1[:, :], op=AL.subtract)
                sc1.append(s1)
                bi1.append(i1)

                w2 = cp.tile([C2, C], F32, name=f"W2q_{l}")
                nc.sync.dma_start(w2[:, :], d_W2[l, :, :])
                emit_q(nc, w2[:, :])
                W2q.append(w2)
                b2t = cp.tile([C, 1], F32, name=f"bias2_{l}")
                nc.sync.dma_start(b2t[:, :], d_b2[l:l + 1, :].rearrange("a b -> b a"))
                emit_q(nc, b2t[:, :])
                nc.vector.tensor_scalar(b2t[:, :], b2t[:, :], QS, MAGIC, AL.mult, AL.add)
                bias2.append(b2t)

            Woq = cp.tile([C, 1], F32, name="Woq")
            nc.sync.dma_start(Woq[:, :], d_Wo[:, :])
            emit_q(nc, Woq[:, :])
            biaso = cp.tile([1, 1], F32, name="biaso")
            nc.sync.dma_start(biaso[:, :], d_bo[:, :])
            emit_q(nc, biaso[:, :])
            nc.vector.tensor_scalar(biaso[:, :], biaso[:, :], QS, MAGIC, AL.mult, AL.add)

            # ---- encoders (x / edge_attr pre-quantized on host) ----
            with (
                tc.tile_pool(name="enc", bufs=2) as enc,
                tc.tile_pool(name="encx", bufs=1) as encx,
                tc.tile_pool(name="ence", bufs=1) as ence,
                tc.tile_pool(name="encps", bufs=2, space="PSUM") as enc_ps,
            ):
                scN4 = replicate4(scN, "scN4", enc_ps)
                biN4 = replicate4(biN, "biN4", enc_ps)
                scE4 = replicate4(scE, "scE4", enc_ps)
                biE4 = replicate4(biE, "biE4", enc_ps)

                t_sb = cp.tile([1, L], F32, name="t_sb")
                nc.sync.dma_start(t_sb[:, :], d_t[:, :])
                t_ps = enc_ps.tile([128, L], F32, name="t_ps", tag="encp")
                nc.tensor.matmul(t_ps[:, :], lhsT=onesr[:, :], rhs=t_sb[:, :],
                                 start=True, stop=True)
                t_bc = cp.tile([128, L], F32, name="t_bc")
                nc.vector.tensor_copy(t_bc[:, :], t_ps[:, :])
                teps_bc = cp.tile([128, L], F32, name="teps_bc")
                nc.vector.tensor_scalar(teps_bc[:, :], t_bc[:, :], GEN_EPS, None,
                                        AL.mult)

                # zero the h pad columns (C:HW) once so the AllGather and
                # dma_gather never move uninitialized DRAM
                zpad = encx.tile([128, TPC, C], F32, name="zpad")
                nc.vector.memset(zpad[:, :, :], 0.0)
                for hbuf in (h_locA, h_locB):
                    nc.sync.dma_start(
                        hbuf[:, C:HW].rearrange("(t p) c -> p t c", p=128),
                        zpad[:, :, :])

                # node encoder
                xseg = encx.tile([XD + 1, NB], F32, name="xseg")
                nc.sync.dma_start(xseg[:, :], d_xT[:, :])
                for b in range(0, TPC, 4):
                    gs = min(4, TPC - b)
                    ep = enc_ps.tile([128, 4 * C], F32, name="encp", tag="encp")
                    for q in range(gs):
                        nc.tensor.matmul(
                            ep[:, q * C:(q + 1) * C],
                            lhsT=xseg[:, (b + q) * 128:(b + q + 1) * 128],
                            rhs=rhs_node[:, :], start=True, stop=True)
                    es = enc.tile([128, 4 * C], F32, name="encs", tag="encs")
                    nc.scalar.activation(es[:, :gs * C], ep[:, :gs * C], ACTF.Copy,
                                         bias=MAGIC, scale=QS)
                    nc.scalar.activation(es[:, :gs * C], es[:, :gs * C], ACTF.Copy,
                                         bias=QB2, scale=QI)
                    nc.vector.tensor_scalar(es[:, :gs * C], es[:, :gs * C],
                                            QMAX, QMIN, AL.min, AL.max)
                    nc.vector.tensor_tensor(es[:, :gs * C], es[:, :gs * C],
                                            scN4[:, :gs * C], op=AL.mult)
                    nc.vector.tensor_tensor(es[:, :gs * C], es[:, :gs * C],
                                            biN4[:, :gs * C], op=AL.add)
                    nc.sync.dma_start(
                        h_locA[b * 128:(b + gs) * 128, 0:C]
                        .rearrange("(t p) c -> p t c", p=128),
                        es[:, :gs * C].rearrange("p (t c) -> p t c", c=C))

                # first AllGather (before the edge encoder so its latency
                # hides behind the edge-encoder work)
                if cfg.use_collectives:
                    nc.gpsimd.collective_compute(
                        "AllGather", AL.bypass, replica_groups=RG,
                        ins=[h_locA[:, :]], outs=[h_fulls[0][:, :]])
                else:
                    for b_ in range(cfg.ncores):
                        nc.sync.dma_start(h_fulls[0][b_ * NB:(b_ + 1) * NB, :],
                                          h_locA[:, :])

                # edge encoder: process per group so e_dram lands group-flat
                # [p][kk][c]; encoder chunk kk covers slots [p*SK+kk] for all p
                # ... wait: group-flat slot = p*SK + kk, so chunk kk is a
                # STRIDED set of flat slots.  eaT is stored in flat slot
                # order; lhsT needs 128 edges (one per PSUM partition) per
                # matmul.  We read eseg as [ED+1, SK*128] for the group and
                # matmul columns [kk*128:(kk+1)*128]?  No: flat slot order is
                # p-major, so columns p*SK+kk.  Use a strided AP instead:
                # lhsT columns for chunk kk = eseg[:, kk::SK] (stride SK).
                eav = d_eaT[:].rearrange("(r e) -> r e", r=ED + 1)
                GFoff = np.concatenate(
                    [[0], np.cumsum(np.asarray(SKg)) * 128]).astype(int)
                for Gi in range(NG):
                    SK = SKg[Gi]
                    base = int(GFoff[Gi])
                    eseg = ence.tile([ED + 1, SKMAX * 128], F32, name="eseg",
                                     tag="eseg", padded_shape=[ED + 1, SKMAX * 128])
                    nc.sync.dma_start(eseg[:, :SK * 128],
                                      eav[:, base:base + SK * 128])
                    esg = eseg[:, :SK * 128].rearrange(
                        "r (p k) -> r p k", p=128)
                    e_grp = e_dram[base * C:(base + SK * 128) * C].rearrange(
                        "(p k c) -> p k c", p=128, k=SK)
                    for b in range(0, SK, 4):
                        gs = min(4, SK - b)
                        ep = enc_ps.tile([128, 4 * C], F32, name="encp", tag="encp")
                        for q in range(gs):
                            nc.tensor.matmul(
                                ep[:, q * C:(q + 1) * C],
                                lhsT=esg[:, :, b + q],
                                rhs=rhs_edge[:, :], start=True, stop=True)
                        es = enc.tile([128, 4 * C], F32, name="encs2", tag="encs")
                        nc.scalar.activation(es[:, :gs * C], ep[:, :gs * C], ACTF.Copy,
                                             bias=MAGIC, scale=QS)
                        nc.scalar.activation(es[:, :gs * C], es[:, :gs * C], ACTF.Copy,
                                             bias=QB2, scale=QI)
                        nc.vector.tensor_scalar(es[:, :gs * C], es[:, :gs * C],
                                                QMAX, QMIN, AL.min, AL.max)
                        nc.vector.tensor_tensor(es[:, :gs * C], es[:, :gs * C],
                                                scE4[:, :gs * C], op=AL.mult)
                        nc.vector.tensor_tensor(es[:, :gs * C], es[:, :gs * C],
                                                biE4[:, :gs * C], op=AL.add)
                        nc.sync.dma_start(
                            e_grp[:, b:b + gs, :],
                            es[:, :gs * C].rearrange("p (t c) -> p t c", c=C))

            # ---- layers ----
            with (
                tc.tile_pool(name="edge", bufs=2) as epool,
                tc.tile_pool(name="node", bufs=2) as npool,
                tc.tile_pool(name="eps", bufs=2, space="PSUM") as ps_edge,
                tc.tile_pool(name="mlp1", bufs=2, space="PSUM") as ps_z1,
                tc.tile_pool(name="mlp2", bufs=1, space="PSUM") as ps_z2,
                tc.tile_pool(name="tr", bufs=2, space="PSUM") as ps_tr,
                tc.tile_pool(name="poolps", bufs=1, space="PSUM") as ps_pool,
            ):
              GFoff = np.concatenate(
                  [[0], np.cumsum(np.asarray(SKg)) * 128]).astype(int)

              for l in range(min(L, cfg.n_layers)):
                  h_in = h_locA if l % 2 == 0 else h_locB
                  h_out = h_locB if l % 2 == 0 else h_locA
                  last = l == L - 1

                  for Gi in range(NG):
                      g0 = Gi * GRP
                      gs = min(GRP, TPC - g0)
                      SK = SKg[Gi]
                      base = int(GFoff[Gi])

                      # --- load the whole group's edge data ---
                      et4 = epool.tile([128, SK, C], F32, name="et4", tag="et",
                                       padded_shape=[128, SKMAX, C])
                      nc.sync.dma_start(
                          et4[:, :, :],
                          e_dram[base * C:(base + SK * 128) * C]
                          .rearrange("(p k c) -> p k c", p=128, k=SK))
                      spans = cfg.gspans[Gi]
                      ic0 = spans[0][2]
                      icw = sum(nj * 8 for (_, nj, _, _) in spans)
                      idxg = epool.tile([128, icw], I16, name="idxg", tag="idxt",
                                        padded_shape=[128, SKMAX * 8])
                      nc.sync.dma_start(idxg[:, :],
                                        d_gidx16[:, ic0:ic0 + icw])
                      dlt4 = epool.tile([128, SK], BF16, name="dlt4", tag="dlt",
                                        padded_shape=[128, SKMAX])
                      nc.sync.dma_start(
                          dlt4[:, :],
                          d_dloc[base:base + SK * 128]
                          .rearrange("(p k) -> p k", p=128))
                      hog = npool.tile([128, GRP, C], F32, name="hog", tag="hog")
                      nc.sync.dma_start(
                          hog[:, :gs, :],
                          h_in[g0 * 128:(g0 + gs) * 128, 0:C]
                          .rearrange("(t p) c -> p t c", p=128))

                      # --- batched h[src] gather: one dma_gather per span of
                      # <=8 chunks (1024 int16 indices), split by src range so
                      # local indices fit int16; gathered rows are 64 fp32
                      # (256B) of which cols 0:C hold h ---
                      hsg4 = epool.tile([128, SK, HW], F32, name="hsg4",
                                        tag="hsg",
                                        padded_shape=[128, SKMAX, HW])
                      for (a, nj, c0, r) in spans:
                          r0 = r * cfg.RMAX
                          r1 = min(r0 + cfg.RMAX, NPAD)
                          nc.gpsimd.dma_gather(
                              out_ap=hsg4[:, a:a + nj, :],
                              in_ap=h_fulls[l][r0:r1, :],
                              idxs_ap=idxg[:, c0 - ic0:c0 - ic0 + nj * 8],
                              num_idxs=nj * 128,
                              num_idxs_reg=nj * 128,
                              elem_size=HW)
                      nc.vector.tensor_tensor(et4[:, :, :], et4[:, :, :],
                                              hsg4[:, :, 0:C], op=AL.add)

                      # r = relu(h_src + e); ex = exp(t*r + t*eps) in bf16;
                      # num-side = ex * r (the +eps on m is folded into h2)
                      nc.scalar.activation(et4[:, :, :], et4[:, :, :], ACTF.Relu)
                      exm4 = epool.tile([128, SK, C2], BF16, name="exm4",
                                        tag="exm",
                                        padded_shape=[128, SKMAX, C2])
                      nc.scalar.activation(exm4[:, :, C:C2], et4[:, :, :],
                                           ACTF.Exp, bias=teps_bc[:, l:l + 1],
                                           scale=t_bc[:, l:l + 1])
                      nc.vector.tensor_copy(exm4[:, :, 0:C], et4[:, :, :])
                      nc.vector.tensor_tensor(exm4[:, :, 0:C], exm4[:, :, 0:C],
                                              exm4[:, :, C:C2], op=AL.mult)

                      # --- per tile: one-hot scatter + h2 + transpose + MLP ---
                      h2qT = npool.tile([C, 512], F32, name="h2qT", tag="h2qT")
                      for t in range(gs):
                          frs = cfg.tfrag[Gi][t]
                          TK = sum(K for _, K in frs)
                          nmm = 0
                          eps_t = ps_edge.tile([128, C2], F32, name="eps_t",
                                               tag="eps_t")
                          for (kk0, K) in frs:
                              for j0 in range(0, K, 4):
                                  jj = min(4, K - j0)
                                  oh4 = epool.tile([128, 4, 128], BF16,
                                                   name="oh4", tag="oh4")
                                  nc.vector.tensor_tensor(
                                      oh4[:, :jj, :],
                                      dlt4[:, kk0 + j0:kk0 + j0 + jj]
                                      .to_broadcast([128, jj, 128]),
                                      iota4b[:, :jj, :], op=AL.is_equal)
                                  for q in range(jj):
                                      nc.tensor.matmul(
                                          eps_t[:, :], lhsT=oh4[:, q, :],
                                          rhs=exm4[:, kk0 + j0 + q, :],
                                          start=(nmm == 0),
                                          stop=(nmm == TK - 1))
                                      nmm += 1

                          # agg = num/max(den,1e-16) + eps;  h2 = h_own + agg
                          dinv = npool.tile([128, C], F32, name="dinv", tag="dinv")
                          nc.vector.tensor_scalar(dinv[:, :], eps_t[:, C:C2],
                                                  1e-16, None, AL.max)
                          nc.vector.reciprocal(dinv[:, :], dinv[:, :])
                          h2 = npool.tile([128, C], F32, name="h2", tag="h2")
                          nc.vector.tensor_tensor(h2[:, :], eps_t[:, 0:C],
                                                  dinv[:, :], op=AL.mult)
                          nc.vector.tensor_scalar(h2[:, :], h2[:, :], GEN_EPS,
                                                  None, AL.add)
                          nc.vector.tensor_tensor(h2[:, :], h2[:, :],
                                                  hog[:, t, :], op=AL.add)
                          emit_q(nc, h2[:, :])
                          trp = ps_tr.tile([C, 128], F32, name="trp", tag="tr")
                          nc.tensor.transpose(trp[:, :], h2[:, :],
                                              identity=ident[:, :])
                          nc.vector.tensor_copy(h2qT[:, t * 128:(t + 1) * 128],
                                                trp[:, :])

                      # --- MLP on the group (up to 4 node tiles) ---
                      w = gs * 128
                      z1p = ps_z1.tile([C2, 512], F32, name="z1p", tag="z1p")
                      nc.tensor.matmul(z1p[:, :w], lhsT=W1q[l][:, :],
                                       rhs=h2qT[:, :w], start=True, stop=True)
                      z1s = npool.tile([C2, 512], F32, name="z1s", tag="z1s")
                      nc.scalar.activation(z1s[:, :w], z1p[:, :w], ACTF.Identity,
                                           bias=bias1[l][:, :], scale=QS)
                      nc.scalar.activation(z1s[:, :w], z1s[:, :w], ACTF.Copy,
                                           bias=QB2, scale=QI)
                      nc.vector.tensor_scalar(z1s[:, :w], z1s[:, :w], QMAX, QMIN,
                                              AL.min, AL.max)
                      nc.scalar.activation(z1s[:, :w], z1s[:, :w], ACTF.Relu,
                                           bias=bi1[l][:, :], scale=sc1[l][:, :])
                      nc.scalar.activation(z1s[:, :w], z1s[:, :w], ACTF.Copy,
                                           bias=MAGIC, scale=QS)
                      nc.scalar.activation(z1s[:, :w], z1s[:, :w], ACTF.Copy,
                                           bias=QB2, scale=QI)
                      nc.vector.tensor_scalar(z1s[:, :w], z1s[:, :w], QMAX, QMIN,
                                              AL.min, AL.max)
                      z2p = ps_z2.tile([C, 512], F32, name="z2p", tag="z2p")
                      nc.tensor.matmul(z2p[:, :w], lhsT=W2q[l][:, :],
                                       rhs=z1s[:, :w], start=True, stop=True)
                      z2s = npool.tile([C, 512], F32, name="z2s", tag="z2s")
                      nc.scalar.activation(z2s[:, :w], z2p[:, :w], ACTF.Identity,
                                           bias=bias2[l][:, :], scale=QS)
                      nc.scalar.activation(z2s[:, :w], z2s[:, :w], ACTF.Copy,
                                           bias=QB2, scale=QI)
                      nc.vector.tensor_scalar(z2s[:, :w], z2s[:, :w], QMAX, QMIN,
                                              AL.min, AL.max)
                      hnext = npool.tile([128, GRP, C], F32, name="hnext",
                                         tag="hnext")
                      for q in range(gs):
                          trq = ps_tr.tile([128, C], F32, name="trq", tag="tr")
                          nc.tensor.transpose(trq[:, :],
                                              z2s[:, q * 128:(q + 1) * 128],
                                              identity=ident[0:C, 0:C])
                          nc.vector.tensor_tensor(hnext[:, q, :], trq[:, :],
                                                  hog[:, q, :], op=AL.add)
                          if last:
                              blt = npool.tile([128, 1], F32, name="blt", tag="blt")
                              nc.sync.dma_start(
                                  blt[:, :],
                                  d_bloc[(g0 + q) * 128:(g0 + q + 1) * 128]
                                  .rearrange("(p one) -> p one", one=1))
                              ohp = npool.tile([128, NW, 128], F32, name="ohp",
                                               tag="ohp")
                              nc.vector.tensor_tensor(
                                  ohp[:, :, :],
                                  blt[:, :].to_broadcast([128, NW, 128]),
                                  iota5[:, :, :], op=AL.is_equal)
                              for wi in range(NW):
                                  pps = ps_pool.tile([128, C], F32, name="pps",
                                                     tag="pps")
                                  nc.tensor.matmul(
                                      pps[:, :], lhsT=ohp[:, wi, :],
                                      rhs=hnext[:, q, :],
                                      start=True, stop=True)
                                  nc.vector.tensor_tensor(
                                      pacc[:, wi, :], pacc[:, wi, :], pps[:, :],
                                      op=AL.add)
                      if not last:
                          nc.sync.dma_start(
                              h_out[g0 * 128:(g0 + gs) * 128, 0:C]
                              .rearrange("(t p) c -> p t c", p=128),
                              hnext[:, :gs, :])

                  if not last:
                      if cfg.use_collectives:
                          nc.gpsimd.collective_compute(
                              "AllGather", AL.bypass, replica_groups=RG,
                              ins=[h_out[:, :]], outs=[h_fulls[l + 1][:, :]])
                      else:
                          for b_ in range(cfg.ncores):
                              nc.sync.dma_start(
                                  h_fulls[l + 1][b_ * NB:(b_ + 1) * NB, :],
                                  h_out[:, :])

              if cfg.n_layers < L:
                  nl = cfg.n_layers
                  hf = h_fulls[min(nl, L - 1)]
                  for b_ in range(NPAD // 128):
                      dbg_t = npool.tile([128, C], F32, name="dbg_t", tag="dbg_t")
                      nc.sync.dma_start(dbg_t[:, :],
                                        hf[b_ * 128:(b_ + 1) * 128, 0:C])
                      nc.sync.dma_start(d_hdbg[b_ * 128:(b_ + 1) * 128, :],
                                        dbg_t[:, :])
                  return nc

              # ---- pooling: write window partials, AllReduce, output head ----
              nc.sync.dma_start(
                  pool_glob[:, :].rearrange("(w p) c -> p w c", p=128),
                  pacc[:, :, :])
              if cfg.use_collectives:
                  nc.gpsimd.collective_compute(
                      "AllReduce", AL.add, replica_groups=RG,
                      ins=[pool_glob[:, :]], outs=[pool_red[:, :]])
              else:
                  nc.sync.dma_start(pool_red[:, :], pool_glob[:, :])

              n_out_tiles = (G + 127) // 128
              for i in range(n_out_tiles):
                  w = min(128, G - i * 128)
                  pt = npool.tile([128, C], F32, name="pt", tag="pt")
                  nc.sync.dma_start(pt[:w, :], pool_red[i * 128:i * 128 + w, :])
                  civ = npool.tile([128, 1], F32, name="civ", tag="civ")
                  nc.sync.dma_start(civ[:w, :],
                                    d_cntinv[i * 128:i * 128 + w].rearrange("(p one) -> p one", one=1))
                  nc.vector.tensor_scalar(pt[:w, :], pt[:w, :], civ[:w, :], None, AL.mult)
                  emit_q(nc, pt[:w, :])
                  trh = ps_tr.tile([C, 128], F32, name="trh", tag="tr")
                  nc.tensor.transpose(trh[:, :w], pt[:w, :], identity=ident[:w, :w])
                  hts = npool.tile([C, 128], F32, name="hts", tag="hts")
                  nc.vector.tensor_copy(hts[:, :w], trh[:, :w])
                  op_ = ps_z2.tile([1, 128], F32, name="op_", tag="z2p")
                  nc.tensor.matmul(op_[:, :w], lhsT=Woq[:, :], rhs=hts[:, :w],
                                   start=True, stop=True)
                  osb = npool.tile([1, 128], F32, name="osb", tag="osb")
                  nc.scalar.activation(osb[:, :w], op_[:, :w], ACTF.Identity,
                                       bias=biaso[:, :], scale=QS)
                  nc.scalar.activation(osb[:, :w], osb[:, :w], ACTF.Copy,
                                       bias=QB2, scale=QI)
                  nc.vector.tensor_scalar(osb[:, :w], osb[:, :w], QMAX, QMIN,
                                          AL.min, AL.max)
                  nc.scalar.activation(osb[:, :w], osb[:, :w], ACTF.Sigmoid)
                  nc.scalar.activation(osb[:, :w], osb[:, :w], ACTF.Copy,
                                       bias=MAGIC, scale=QS)
                  nc.scalar.activation(osb[:, :w], osb[:, :w], ACTF.Copy,
                                       bias=QB2, scale=QI)
                  nc.sync.dma_start(
                      d_out[i * 128:i * 128 + w, :].rearrange("w one -> one w"),
                      osb[:, :w])

    return nc


# ----------------------------------------------------------------------------
# Entry point.
# ----------------------------------------------------------------------------

def run(inputs, cfg, **run_kwargs):
    global LAST_RESULTS
    in_maps = preprocess(inputs, cfg)
    nc = build(cfg)
    if not nc.is_finalized():
        nc.finalize()
    res = run_bass_kernel_spmd(nc, in_maps, core_ids=list(range(cfg.ncores)),
                               **run_kwargs)
    LAST_RESULTS = res
    return res.results[0]["out"].reshape(cfg.G, 1).astype(np.float32)


def kernel(**inputs) -> np.ndarray:
    cfg = Cfg(N=100000, E=3200000, G=512, XD=8, ED=4, C=32, L=4)
    return run(inputs, cfg)


# revision 28
# speedup vs baseline: 1.3037x; 1.0391x over previous
"""Trainium2 Bass kernel: nn_BV_Model (GENConv GNN, softmax aggregation, 4 layers).

Strategy (8 NeuronCores, SPMD):
  - Nodes are partitioned into 8 contiguous blocks (12544/core, padded).
  - Edges are sorted by destination node and bucketed per destination
    node-tile (128 nodes); each core owns the edges whose dst falls in its
    block.  Tiles are processed in groups of 4; within a group the edge
    slots are laid out "group-flat" [p][kk][c] (p = SBUF partition,
    kk = chunk slot within the group) so each group's edge features /
    src indices / dst offsets load as one large DMA and h[src] is fetched
    with ONE batched indirect DMA (~13k descriptors) per group.
  - Per layer: m = relu(h[src]+e), ex = exp(t*m) (bf16), and the segment
    softmax numerator/denominator reduce edges->nodes with one-hot(dst)
    matmuls (bf16) accumulated in PSUM.  No segment-max: s_max ~ 65,
    exp fits fp32/bf16 range (verified offline).  Node MLP runs on the
    tensor engine in transposed layout.  h is AllGathered per layer.
  - Global mean pool via one-hot(graph) matmuls, AllReduce, output head.

Fake-quantization q(x) = clip(rne(x*1024), -32768, 32767)/1024 is computed
exactly with the round-to-nearest-even "magic number" trick (+1.5*2^23).
x/edge_attr are pre-quantized on the host (same RNE semantics).
"""

import os
os.environ.setdefault("MYCRO_LOCAL_CACHE", "1")

import math
import numpy as np
import ml_dtypes

import concourse.bacc as bacc
import concourse.tile as tile
import concourse.bass as bass
from concourse import mybir
from concourse.bass import IndirectOffsetOnAxis
from concourse.bass_utils import run_bass_kernel_spmd

F32 = mybir.dt.float32
BF16 = mybir.dt.bfloat16
I32 = mybir.dt.int32
ACTF = mybir.ActivationFunctionType
AL = mybir.AluOpType

MAGIC = 12582912.0           # 1.5*2^23 : fp32 RNE rounding magic
QS = 1024.0                  # 2^10
QI = 1.0 / 1024.0
QB2 = -12288.0               # -MAGIC * 2^-10
QMAX = 32767.0 / 1024.0
QMIN = -32.0
GEN_EPS = 1e-7
BN_EPS = 1e-5
NCORES = 8
GRP = 4                      # node tiles per group (shared w/ MLP batching)

LAST_RESULTS = None          # BassKernelResults of the most recent run (for test.py)


class Cfg:
    def __init__(self, N, E, G, XD=8, ED=4, C=32, L=4, ncores=NCORES,
                 use_collectives=True, use_shared=True,
                 gather_acc=False, gather_max_cols=64, n_layers=None):
        self.N, self.E, self.G = N, E, G
        self.XD, self.ED, self.C, self.L = XD, ED, C, L
        self.ncores = ncores
        self.use_collectives = use_collectives and ncores > 1
        self.use_shared = use_shared
        self.gather_acc = gather_acc
        self.gather_max_cols = gather_max_cols
        self.n_layers = L if n_layers is None else n_layers
        self.TPC = (N + ncores * 128 - 1) // (ncores * 128)    # node tiles per core
        self.NB = self.TPC * 128                               # nodes per core (padded)
        self.NPAD = self.NB * ncores
        self.PG = ((G + 128) + 127) // 128 * 128               # pooled scatter rows
        self.SKg = None                                        # chunks per group [NG]
        self.E_PAD = None                                      # padded edges per core
        self.tfrag = None                                      # per group tile chunk frags
        self.gspans = None                                     # per group gather spans
        self.IDXW = None
        self.NR = None
        self.RMAX = 32768


def qnp(a):
    """Host-side ap_fixed<16,6> fake quantization (RNE, matches HW magic)."""
    y = np.round(a.astype(np.float64) * QS) * QI
    return np.clip(y, QMIN, QMAX).astype(np.float32)


# ----------------------------------------------------------------------------
# Host-side preprocessing: sort/bucket edges, build per-core input arrays.
# ----------------------------------------------------------------------------

def preprocess(inputs, cfg):
    x = qnp(np.asarray(inputs["x"], np.float32))
    ea = qnp(np.asarray(inputs["edge_attr"], np.float32))
    ei = np.asarray(inputs["edge_index"]).astype(np.int64)
    batch = np.asarray(inputs["batch"]).astype(np.int64)
    N, E, G = cfg.N, cfg.E, cfg.G
    XD, ED, C, L = cfg.XD, cfg.ED, cfg.C, cfg.L
    TPC, NB = cfg.TPC, cfg.NB
    NC_ = cfg.ncores

    src, dst = ei[0], ei[1]
    # sort by (dst tile, src): tile bucketing unchanged, but edges within a
    # tile then gather ascending clustered h rows (HBM locality)
    order = np.argsort((dst // 128).astype(np.int64) * (N + 1) + src,
                       kind="stable")
    src_s = src[order]
    dst_s = dst[order]
    ea_s = ea[order]

    ntiles = NC_ * TPC
    bnd = np.searchsorted(dst_s, np.arange(ntiles + 1) * 128)
    NG = (TPC + GRP - 1) // GRP
    RMAX = 32768
    NR = (cfg.NPAD + RMAX - 1) // RMAX           # src ranges (int16 index cap)
    GCH = 8                                      # chunks per dma_gather (1024 idx)

    # per (core, tile, range) edge counts (ranges contiguous: src-sorted)
    cnt_r = np.zeros((NC_, TPC, NR), np.int64)
    roff = np.zeros((NC_, TPC, NR), np.int64)
    for c in range(NC_):
        for g in range(TPC):
            tl = c * TPC + g
            seg = src_s[bnd[tl]:bnd[tl + 1]]
            rb = np.searchsorted(seg, np.arange(1, NR + 1) * RMAX)
            prev = 0
            for r in range(NR):
                cnt_r[c, g, r] = rb[r] - prev
                roff[c, g, r] = prev
                prev = rb[r]
    K_gr = ((cnt_r + 127) // 128).max(axis=0)    # [TPC, NR] shared chunks
    for g in range(TPC):
        if K_gr[g].sum() == 0:
            K_gr[g, 0] = 1                       # >=1 chunk per tile
    SKg = np.array([int(K_gr[g0:g0 + GRP].sum())
                    for g0 in range(0, TPC, GRP)], np.int64)  # [NG]
    E_PAD = int(K_gr.sum()) * 128
    GFoff = np.concatenate([[0], np.cumsum(SKg) * 128])

    # group slot order (range r, tile t, chunk j); flat = base + p*SK + kk
    rspan = []                                   # per group: [(kk0, kk1)]*NR
    tfrag = []                                   # per group: t -> [(kk0, K)]
    for Gi in range(NG):
        g0 = Gi * GRP
        gs = min(GRP, TPC - g0)
        rs, tf, nck = [], [[] for _ in range(gs)], 0
        for r in range(NR):
            kk0 = nck
            for t in range(gs):
                K = int(K_gr[g0 + t, r])
                if K:
                    tf[t].append((nck, K))
                nck += K
            rs.append((kk0, nck))
        rspan.append(rs)
        tfrag.append(tf)

    gspans = []                                  # per group: [(kk0, nj, col0, r)]
    col = 0
    for Gi in range(NG):
        spans = []
        for r in range(NR):
            kk0, kk1 = rspan[Gi][r]
            a = kk0
            while a < kk1:
                nj = min(GCH, kk1 - a)
                spans.append((a, nj, col, r))
                col += nj * 8
                a += nj
        gspans.append(spans)
    IDXW = col                                   # int16 idx cols (16-wrapped)

    idx16_a = np.zeros((NC_, 128, IDXW), np.int16)
    dloc_a = np.full((NC_, E_PAD), -1.0, ml_dtypes.bfloat16)
    eaT_a = np.zeros((NC_, ED + 1, E_PAD), np.float32)
    eaT_a[:, ED, :] = 1.0
    for c in range(NC_):
        for Gi in range(NG):
            g0 = Gi * GRP
            gs = min(GRP, TPC - g0)
            SK = int(SKg[Gi])
            base = int(GFoff[Gi])
            loc_idx = np.zeros((128, SK), np.int64)
            kkc = 0
            for r in range(NR):
                for t in range(gs):
                    g = g0 + t
                    tl = c * TPC + g
                    K = int(K_gr[g, r])
                    if K == 0:
                        continue
                    m = int(cnt_r[c, g, r])
                    if m > 0:
                        i_ar = np.arange(m)
                        rows = bnd[tl] + int(roff[c, g, r]) + i_ar
                        pp = i_ar % 128
                        kk = kkc + i_ar // 128
                        flat = base + pp * SK + kk
                        dloc_a[c, flat] = (dst_s[rows] - tl * 128
                                           ).astype(np.float32)
                        # advanced (flat) axis comes FIRST -> assign [m, ED]
                        eaT_a[c, :ED, flat] = ea_s[rows]
                        loc_idx[pp, kk] = src_s[rows] - r * RMAX
                    kkc += K
            for (a, nj, c0, r) in gspans[Gi]:
                ni = nj * 128
                i_ar = np.arange(ni)
                vals = loc_idx[i_ar % 128, a + i_ar // 128]
                blk = np.zeros((16, nj * 8), np.int16)
                blk[i_ar % 16, i_ar // 16] = vals.astype(np.int16)
                idx16_a[c, :, c0:c0 + nj * 8] = np.tile(blk, (8, 1))

    xT_a = np.zeros((NC_, XD + 1, NB), np.float32)
    xT_a[:, XD, :] = 1.0
    bloc_a = np.full((NC_, NB), -1.0, np.float32)
    first_g = np.zeros(NC_, np.int64)
    for c in range(NC_):
        lo, hi = c * NB, min((c + 1) * NB, N)
        xT_a[c, :XD, : hi - lo] = x[lo:hi].T
        first_g[c] = batch[lo]
        assert batch[hi - 1] - batch[lo] < 128, "graph window exceeds 128"
        bloc_a[c, : hi - lo] = batch[lo:hi].astype(np.float32)

    prow_a = (first_g[:, None] + np.arange(128)[None, :]).astype(np.int32)
    assert prow_a.max() < cfg.PG
    cnt_g = np.bincount(batch, minlength=G).astype(np.float32)
    cnt_inv = np.zeros(cfg.PG, np.float32)
    cnt_inv[:G] = np.float32(1.0) / np.maximum(cnt_g, np.float32(1.0))

    cfg.SKg = [int(k) for k in SKg]
    cfg.E_PAD = E_PAD
    cfg.tfrag = tfrag
    cfg.gspans = gspans
    cfg.IDXW = IDXW
    cfg.NR = NR
    cfg.RMAX = RMAX

    def f32(a):
        return np.ascontiguousarray(np.asarray(a, np.float32))

    shared = dict(
        W_node=f32(inputs["W_node"]), b_node=f32(inputs["b_node"]).reshape(1, C),
        W_edge=f32(inputs["W_edge"]), b_edge=f32(inputs["b_edge"]).reshape(1, C),
        bnn_g=f32(inputs["bnn_g"]).reshape(1, C), bnn_b=f32(inputs["bnn_b"]).reshape(1, C),
        bnn_m=f32(inputs["bnn_m"]).reshape(1, C), bnn_v=f32(inputs["bnn_v"]).reshape(1, C),
        bne_g=f32(inputs["bne_g"]).reshape(1, C), bne_b=f32(inputs["bne_b"]).reshape(1, C),
        bne_m=f32(inputs["bne_m"]).reshape(1, C), bne_v=f32(inputs["bne_v"]).reshape(1, C),
        t=f32(inputs["t"]).reshape(1, L),
        W1=f32(inputs["W1"]), b1=f32(inputs["b1"]),
        bn1_g=f32(inputs["bn1_g"]), bn1_b=f32(inputs["bn1_b"]),
        bn1_m=f32(inputs["bn1_m"]), bn1_v=f32(inputs["bn1_v"]),
        W2=f32(inputs["W2"]), b2=f32(inputs["b2"]),
        W_out=f32(inputs["W_out"]), b_out=f32(inputs["b_out"]).reshape(1, 1),
        cnt_inv=cnt_inv,
    )
    in_maps = []
    for c in range(NC_):
        im = dict(shared)
        im.update(
            xT=xT_a[c], eaT=eaT_a[c].reshape(-1), gidx16=idx16_a[c],
            dloc=dloc_a[c], bloc=bloc_a[c], prow=prow_a[c],
        )
        in_maps.append(im)
    return in_maps


# ----------------------------------------------------------------------------
# Device program.
# ----------------------------------------------------------------------------

def emit_q(nc, ap, pre_bias_ap=None, clip=True):
    """In-place fake quantization of `ap` (fp32): q(x) (+fused bias if given).

    If pre_bias_ap is given it must hold (1024*bias_q + MAGIC) per partition and
    the op computes q(x + bias_q)."""
    if pre_bias_ap is None:
        nc.scalar.activation(ap, ap, ACTF.Copy, bias=MAGIC, scale=QS)
    else:
        nc.scalar.activation(ap, ap, ACTF.Identity, bias=pre_bias_ap, scale=QS)
    nc.scalar.activation(ap, ap, ACTF.Copy, bias=QB2, scale=QI)
    if clip:
        nc.vector.tensor_scalar(ap, ap, QMAX, QMIN, AL.min, AL.max)


def build(cfg):
    C, L, TPC, NB = cfg.C, cfg.L, cfg.TPC, cfg.NB
    XD, ED, G, PG = cfg.XD, cfg.ED, cfg.G, cfg.PG
    NPAD, E_PAD, SKg = cfg.NPAD, cfg.E_PAD, cfg.SKg
    HW = 2 * C                      # h row width in DRAM (256B for dma_gather)
    C2 = 2 * C
    NG = (TPC + GRP - 1) // GRP
    SKMAX = max(SKg)
    RG = [list(range(cfg.ncores))]
    SHARED = "Shared" if (cfg.use_shared and cfg.use_collectives) else "Local"

    nc = bacc.Bacc("TRN2", target_bir_lowering=False, debug=False,
                   enable_asserts=False, num_devices=cfg.ncores)

    # ---- kernel I/O ----
    d_xT = nc.dram_tensor("xT", [XD + 1, NB], F32, kind="ExternalInput")
    d_eaT = nc.dram_tensor("eaT", [(ED + 1) * E_PAD], F32, kind="ExternalInput")
    I16 = mybir.dt.int16
    d_gidx16 = nc.dram_tensor("gidx16", [128, cfg.IDXW], I16,
                              kind="ExternalInput")
    d_dloc = nc.dram_tensor("dloc", [E_PAD], BF16, kind="ExternalInput")
    d_bloc = nc.dram_tensor("bloc", [NB], F32, kind="ExternalInput")
    d_prow = nc.dram_tensor("prow", [128], I32, kind="ExternalInput")
    d_cntinv = nc.dram_tensor("cnt_inv", [PG], F32, kind="ExternalInput")
    d_Wn = nc.dram_tensor("W_node", [XD, C], F32, kind="ExternalInput")
    d_bn_ = nc.dram_tensor("b_node", [1, C], F32, kind="ExternalInput")
    d_We = nc.dram_tensor("W_edge", [ED, C], F32, kind="ExternalInput")
    d_be = nc.dram_tensor("b_edge", [1, C], F32, kind="ExternalInput")
    d_bnr = {k: nc.dram_tensor(k, [1, C], F32, kind="ExternalInput")
             for k in ["bnn_g", "bnn_b", "bnn_m", "bnn_v",
                       "bne_g", "bne_b", "bne_m", "bne_v"]}
    d_t = nc.dram_tensor("t", [1, L], F32, kind="ExternalInput")
    d_W1 = nc.dram_tensor("W1", [L, C, C2], F32, kind="ExternalInput")
    d_b1 = nc.dram_tensor("b1", [L, C2], F32, kind="ExternalInput")
    d_bn1 = {k: nc.dram_tensor(k, [L, C2], F32, kind="ExternalInput")
             for k in ["bn1_g", "bn1_b", "bn1_m", "bn1_v"]}
    d_W2 = nc.dram_tensor("W2", [L, C2, C], F32, kind="ExternalInput")
    d_b2 = nc.dram_tensor("b2", [L, C], F32, kind="ExternalInput")
    d_Wo = nc.dram_tensor("W_out", [C, 1], F32, kind="ExternalInput")
    d_bo = nc.dram_tensor("b_out", [1, 1], F32, kind="ExternalInput")
    d_out = nc.dram_tensor("out", [G, 1], F32, kind="ExternalOutput")
    d_hdbg = nc.dram_tensor("h_dbg", [NPAD, C], F32, kind="ExternalOutput")

    # ---- inline constants ----
    eye = np.eye(128, dtype=np.float32)
    iota4_np = np.tile(np.arange(128, dtype=np.float32), (128, 4, 1))
    ones_np = np.ones((1, 128), np.float32)
    c_eye = nc.inline_tensor(eye, "c_eye")
    c_iota4b = nc.inline_tensor(iota4_np.astype(ml_dtypes.bfloat16), "c_iota4b")
    NW = PG // 128                                   # pooling windows
    iota5_np = (np.tile(np.arange(128, dtype=np.float32), (128, NW, 1))
                + (np.arange(NW, dtype=np.float32) * 128)[None, :, None])
    c_iota5 = nc.inline_tensor(iota5_np, "c_iota5")
    c_ones = nc.inline_tensor(ones_np, "c_ones")

    with tile.TileContext(nc) as tc:
        with (
            tc.tile_pool(name="dram", bufs=1, space="DRAM") as dpool,
            tc.tile_pool(name="const", bufs=1) as cp,
        ):
            # ---- internal DRAM ----
            # h rows are 64 fp32 wide (256B) so dma_gather's elem-size
            # constraint holds; only cols 0:C are meaningful
            h_locA = dpool.tile([NB, HW], F32, name="h_locA")
            h_locB = dpool.tile([NB, HW], F32, name="h_locB")
            h_fulls = [dpool.tile([NPAD, HW], F32, addr_space=SHARED,
                                  name=f"h_full_{l}") for l in range(L)]
            e_dram = dpool.tile([E_PAD * C], F32, name="e_dram")
            pool_glob = dpool.tile([PG, C], F32, name="pool_glob")
            pool_red = dpool.tile([PG, C], F32, addr_space=SHARED, name="pool_red")

            # ---- constants to SBUF ----
            ident = cp.tile([128, 128], F32, name="ident")
            nc.sync.dma_start(ident[:, :], c_eye[:, :])
            iota4b = cp.tile([128, 4, 128], BF16, name="iota4b")
            nc.sync.dma_start(iota4b[:, :, :], c_iota4b[:, :, :])
            iota5 = cp.tile([128, NW, 128], F32, name="iota5")
            nc.sync.dma_start(iota5[:, :, :], c_iota5[:, :, :])
            pacc = cp.tile([128, NW, C], F32, name="pacc")
            nc.vector.memset(pacc[:, :, :], 0.0)
            onesr = cp.tile([1, 128], F32, name="onesr")
            nc.sync.dma_start(onesr[:, :], c_ones[:, :])

            # ---- parameter prep ----
            rhs_node = cp.tile([XD + 1, C], F32, name="rhs_node")
            nc.sync.dma_start(rhs_node[:XD, :], d_Wn[:, :])
            nc.sync.dma_start(rhs_node[XD:XD + 1, :], d_bn_[:, :])
            emit_q(nc, rhs_node[:, :])
            rhs_edge = cp.tile([ED + 1, C], F32, name="rhs_edge")
            nc.sync.dma_start(rhs_edge[:ED, :], d_We[:, :])
            nc.sync.dma_start(rhs_edge[ED:ED + 1, :], d_be[:, :])
            emit_q(nc, rhs_edge[:, :])

            def bn_rows(pref):
                g_ = cp.tile([1, C], F32, name=pref + "_g")
                b_ = cp.tile([1, C], F32, name=pref + "_b")
                m_ = cp.tile([1, C], F32, name=pref + "_m")
                sc = cp.tile([1, C], F32, name=pref + "_sc")
                bi = cp.tile([1, C], F32, name=pref + "_bi")
                nc.sync.dma_start(g_[:, :], d_bnr[pref + "_g"][:, :])
                nc.sync.dma_start(b_[:, :], d_bnr[pref + "_b"][:, :])
                nc.sync.dma_start(m_[:, :], d_bnr[pref + "_m"][:, :])
                nc.sync.dma_start(sc[:, :], d_bnr[pref + "_v"][:, :])
                nc.vector.tensor_scalar(sc[:, :], sc[:, :], BN_EPS, None, AL.add)
                nc.scalar.activation(sc[:, :], sc[:, :], ACTF.Sqrt)
                nc.vector.reciprocal(sc[:, :], sc[:, :])
                nc.vector.tensor_tensor(sc[:, :], sc[:, :], g_[:, :], op=AL.mult)
                nc.vector.tensor_tensor(bi[:, :], m_[:, :], sc[:, :], op=AL.mult)
                nc.vector.tensor_tensor(bi[:, :], b_[:, :], bi[:, :], op=AL.subtract)
                return sc, bi

            scN, biN = bn_rows("bnn")
            scE, biE = bn_rows("bne")

            def replicate4(row, nm, pool):
                ps = pool.tile([128, C], F32, name="rep_ps", tag="encp")
                nc.tensor.matmul(ps[:, :], lhsT=onesr[:, :], rhs=row[:, :],
                                 start=True, stop=True)
                out4 = cp.tile([128, 4 * C], F32, name=nm)
                for q in range(4):
                    nc.vector.tensor_copy(out4[:, q * C:(q + 1) * C], ps[:, :])
                return out4

            W1q, bias1, sc1, bi1, W2q, bias2 = [], [], [], [], [], []
            for l in range(L):
                w1 = cp.tile([C, C2], F32, name=f"W1q_{l}")
                nc.sync.dma_start(w1[:, :], d_W1[l, :, :])
                emit_q(nc, w1[:, :])
                W1q.append(w1)
                b1t = cp.tile([C2, 1], F32, name=f"bias1_{l}")
                nc.sync.dma_start(b1t[:, :], d_b1[l:l + 1, :].rearrange("a b -> b a"))
                emit_q(nc, b1t[:, :])
                nc.vector.tensor_scalar(b1t[:, :], b1t[:, :], QS, MAGIC, AL.mult, AL.add)
                bias1.append(b1t)

                g1 = cp.tile([C2, 1], F32, name=f"g1_{l}")
                bb1 = cp.tile([C2, 1], F32, name=f"bb1_{l}")
                m1 = cp.tile([C2, 1], F32, name=f"m1_{l}")
                s1 = cp.tile([C2, 1], F32, name=f"sc1_{l}")
                i1 = cp.tile([C2, 1], F32, name=f"bi1_{l}")
                nc.sync.dma_start(g1[:, :], d_bn1["bn1_g"][l:l + 1, :].rearrange("a b -> b a"))
                nc.sync.dma_start(bb1[:, :], d_bn1["bn1_b"][l:l + 1, :].rearrange("a b -> b a"))
                nc.sync.dma_start(m1[:, :], d_bn1["bn1_m"][l:l + 1, :].rearrange("a b -> b a"))
                nc.sync.dma_start(s1[:, :], d_bn1["bn1_v"][l:l + 1, :].rearrange("a b -> b a"))
                nc.vector.tensor_scalar(s1[:, :], s1[:, :], BN_EPS, None, AL.add)
                nc.scalar.activation(s1[:, :], s1[:, :], ACTF.Sqrt)
                nc.vector.reciprocal(s1[:, :], s1[:, :])
                nc.vector.tensor_tensor(s1[:, :], s1[:, :], g1[:, :], op=AL.mult)
                nc.vector.tensor_tensor(i1[:, :], m1[:, :], s1[:, :], op=AL.mult)
                nc.vector.tensor_tensor(i1[:, :], bb1[:, :], i1[:, :], op=AL.subtract)
                sc1.append(s1)
                bi1.append(i1)

                w2 = cp.tile([C2, C], F32, name=f"W2q_{l}")
                nc.sync.dma_start(w2[:, :], d_W2[l, :, :])
                emit_q(nc, w2[:, :])
                W2q.append(w2)
                b2t = cp.tile([C, 1], F32, name=f"bias2_{l}")
                nc.sync.dma_start(b2t[:, :], d_b2[l:l + 1, :].rearrange("a b -> b a"))
                emit_q(nc, b2t[:, :])
                nc.vector.tensor_scalar(b2t[:, :], b2t[:, :], QS, MAGIC, AL.mult, AL.add)
                bias2.append(b2t)

            Woq = cp.tile([C, 1], F32, name="Woq")
            nc.sync.dma_start(Woq[:, :], d_Wo[:, :])
            emit_q(nc, Woq[:, :])
            biaso = cp.tile([1, 1], F32, name="biaso")
            nc.sync.dma_start(biaso[:, :], d_bo[:, :])
            emit_q(nc, biaso[:, :])
            nc.vector.tensor_scalar(biaso[:, :], biaso[:, :], QS, MAGIC, AL.mult, AL.add)

            # ---- encoders (x / edge_attr pre-quantized on host) ----
            with (
                tc.tile_pool(name="enc", bufs=2) as enc,
                tc.tile_pool(name="encx", bufs=1) as encx,
                tc.tile_pool(name="ence", bufs=1) as ence,
                tc.tile_pool(name="encps", bufs=2, space="PSUM") as enc_ps,
            ):
                scN4 = replicate4(scN, "scN4", enc_ps)
                biN4 = replicate4(biN, "biN4", enc_ps)
                scE4 = replicate4(scE, "scE4", enc_ps)
                biE4 = replicate4(biE, "biE4", enc_ps)

                t_sb = cp.tile([1, L], F32, name="t_sb")
                nc.sync.dma_start(t_sb[:, :], d_t[:, :])
                t_ps = enc_ps.tile([128, L], F32, name="t_ps", tag="encp")
                nc.tensor.matmul(t_ps[:, :], lhsT=onesr[:, :], rhs=t_sb[:, :],
                                 start=True, stop=True)
                t_bc = cp.tile([128, L], F32, name="t_bc")
                nc.vector.tensor_copy(t_bc[:, :], t_ps[:, :])
                teps_bc = cp.tile([128, L], F32, name="teps_bc")
                nc.vector.tensor_scalar(teps_bc[:, :], t_bc[:, :], GEN_EPS, None,
                                        AL.mult)

                # zero the h pad columns (C:HW) once so the AllGather and
                # dma_gather never move uninitialized DRAM
                zpad = encx.tile([128, TPC, C], F32, name="zpad")
                nc.vector.memset(zpad[:, :, :], 0.0)
                for hbuf in (h_locA, h_locB):
                    nc.sync.dma_start(
                        hbuf[:, C:HW].rearrange("(t p) c -> p t c", p=128),
                        zpad[:, :, :])

                # node encoder
                xseg = encx.tile([XD + 1, NB], F32, name="xseg")
                nc.sync.dma_start(xseg[:, :], d_xT[:, :])
                for b in range(0, TPC, 4):
                    gs = min(4, TPC - b)
                    ep = enc_ps.tile([128, 4 * C], F32, name="encp", tag="encp")
                    for q in range(gs):
                        nc.tensor.matmul(
                            ep[:, q * C:(q + 1) * C],
                            lhsT=xseg[:, (b + q) * 128:(b + q + 1) * 128],
                            rhs=rhs_node[:, :], start=True, stop=True)
                    es = enc.tile([128, 4 * C], F32, name="encs", tag="encs")
                    nc.scalar.activation(es[:, :gs * C], ep[:, :gs * C], ACTF.Copy,
                                         bias=MAGIC, scale=QS)
                    nc.scalar.activation(es[:, :gs * C], es[:, :gs * C], ACTF.Copy,
                                         bias=QB2, scale=QI)
                    nc.vector.tensor_scalar(es[:, :gs * C], es[:, :gs * C],
                                            QMAX, QMIN, AL.min, AL.max)
                    nc.vector.tensor_tensor(es[:, :gs * C], es[:, :gs * C],
                                            scN4[:, :gs * C], op=AL.mult)
                    nc.vector.tensor_tensor(es[:, :gs * C], es[:, :gs * C],
                                            biN4[:, :gs * C], op=AL.add)
                    nc.sync.dma_start(
                        h_locA[b * 128:(b + gs) * 128, 0:C]
                        .rearrange("(t p) c -> p t c", p=128),
                        es[:, :gs * C].rearrange("p (t c) -> p t c", c=C))

                # first AllGather (before the edge encoder so its latency
                # hides behind the edge-encoder work)
                if cfg.use_collectives:
                    nc.gpsimd.collective_compute(
                        "AllGather", AL.bypass, replica_groups=RG,
                        ins=[h_locA[:, :]], outs=[h_fulls[0][:, :]])
                else:
                    for b_ in range(cfg.ncores):
                        nc.sync.dma_start(h_fulls[0][b_ * NB:(b_ + 1) * NB, :],
                                          h_locA[:, :])

                # edge encoder: process per group so e_dram lands group-flat
                # [p][kk][c]; encoder chunk kk covers slots [p*SK+kk] for all p
                # ... wait: group-flat slot = p*SK + kk, so chunk kk is a
                # STRIDED set of flat slots.  eaT is stored in flat slot
                # order; lhsT needs 128 edges (one per PSUM partition) per
                # matmul.  We read eseg as [ED+1, SK*128] for the group and
                # matmul columns [kk*128:(kk+1)*128]?  No: flat slot order is
                # p-major, so columns p*SK+kk.  Use a strided AP instead:
                # lhsT columns for chunk kk = eseg[:, kk::SK] (stride SK).
                eav = d_eaT[:].rearrange("(r e) -> r e", r=ED + 1)
                GFoff = np.concatenate(
                    [[0], np.cumsum(np.asarray(SKg)) * 128]).astype(int)
                for Gi in range(NG):
                    SK = SKg[Gi]
                    base = int(GFoff[Gi])
                    eseg = ence.tile([ED + 1, SKMAX * 128], F32, name="eseg",
                                     tag="eseg", padded_shape=[ED + 1, SKMAX * 128])
                    nc.sync.dma_start(eseg[:, :SK * 128],
                                      eav[:, base:base + SK * 128])
                    esg = eseg[:, :SK * 128].rearrange(
                        "r (p k) -> r p k", p=128)
                    e_grp = e_dram[base * C:(base + SK * 128) * C].rearrange(
                        "(p k c) -> p k c", p=128, k=SK)
                    for b in range(0, SK, 4):
                        gs = min(4, SK - b)
                        ep = enc_ps.tile([128, 4 * C], F32, name="encp", tag="encp")
                        for q in range(gs):
                            nc.tensor.matmul(
                                ep[:, q * C:(q + 1) * C],
                                lhsT=esg[:, :, b + q],
                                rhs=rhs_edge[:, :], start=True, stop=True)
                        es = enc.tile([128, 4 * C], F32, name="encs2", tag="encs")
                        nc.scalar.activation(es[:, :gs * C], ep[:, :gs * C], ACTF.Copy,
                                             bias=MAGIC, scale=QS)
                        nc.scalar.activation(es[:, :gs * C], es[:, :gs * C], ACTF.Copy,
                                             bias=QB2, scale=QI)
                        nc.vector.tensor_scalar(es[:, :gs * C], es[:, :gs * C],
                                                QMAX, QMIN, AL.min, AL.max)
                        nc.vector.tensor_tensor(es[:, :gs * C], es[:, :gs * C],
                                                scE4[:, :gs * C], op=AL.mult)
                        nc.vector.tensor_tensor(es[:, :gs * C], es[:, :gs * C],
                                                biE4[:, :gs * C], op=AL.add)
                        nc.sync.dma_start(
                            e_grp[:, b:b + gs, :],
                            es[:, :gs * C].rearrange("p (t c) -> p t c", c=C))

            # ---- layers ----
            with (
                tc.tile_pool(name="edge", bufs=2) as epool,
                tc.tile_pool(name="node", bufs=2) as npool,
                tc.tile_pool(name="eps", bufs=2, space="PSUM") as ps_edge,
                tc.tile_pool(name="mlp1", bufs=2, space="PSUM") as ps_z1,
                tc.tile_pool(name="mlp2", bufs=1, space="PSUM") as ps_z2,
                tc.tile_pool(name="tr", bufs=2, space="PSUM") as ps_tr,
                tc.tile_pool(name="poolps", bufs=1, space="PSUM") as ps_pool,
            ):
              GFoff = np.concatenate(
                  [[0], np.cumsum(np.asarray(SKg)) * 128]).astype(int)

              for l in range(min(L, cfg.n_layers)):
                  h_in = h_locA if l % 2 == 0 else h_locB
                  h_out = h_locB if l % 2 == 0 else h_locA
                  last = l == L - 1

                  for Gi in range(NG):
                      g0 = Gi * GRP
                      gs = min(GRP, TPC - g0)
                      SK = SKg[Gi]
                      base = int(GFoff[Gi])

                      # --- load the whole group's edge data ---
                      et4 = epool.tile([128, SK, C], F32, name="et4", tag="et",
                                       padded_shape=[128, SKMAX, C])
                      nc.sync.dma_start(
                          et4[:, :, :],
                          e_dram[base * C:(base + SK * 128) * C]
                          .rearrange("(p k c) -> p k c", p=128, k=SK))
                      spans = cfg.gspans[Gi]
                      ic0 = spans[0][2]
                      icw = sum(nj * 8 for (_, nj, _, _) in spans)
                      idxg = epool.tile([128, icw], I16, name="idxg", tag="idxt",
                                        padded_shape=[128, SKMAX * 8])
                      nc.sync.dma_start(idxg[:, :],
                                        d_gidx16[:, ic0:ic0 + icw])
                      dlt4 = epool.tile([128, SK], BF16, name="dlt4", tag="dlt",
                                        padded_shape=[128, SKMAX])
                      nc.sync.dma_start(
                          dlt4[:, :],
                          d_dloc[base:base + SK * 128]
                          .rearrange("(p k) -> p k", p=128))
                      hog = npool.tile([128, GRP, C], F32, name="hog", tag="hog")
                      nc.sync.dma_start(
                          hog[:, :gs, :],
                          h_in[g0 * 128:(g0 + gs) * 128, 0:C]
                          .rearrange("(t p) c -> p t c", p=128))

                      # --- batched h[src] gather: one dma_gather per span of
                      # <=8 chunks (1024 int16 indices), split by src range so
                      # local indices fit int16; gathered rows are 64 fp32
                      # (256B) of which cols 0:C hold h ---
                      hsg4 = epool.tile([128, SK, HW], F32, name="hsg4",
                                        tag="hsg",
                                        padded_shape=[128, SKMAX, HW])
                      for (a, nj, c0, r) in spans:
                          r0 = r * cfg.RMAX
                          r1 = min(r0 + cfg.RMAX, NPAD)
                          nc.gpsimd.dma_gather(
                              out_ap=hsg4[:, a:a + nj, :],
                              in_ap=h_fulls[l][r0:r1, :],
                              idxs_ap=idxg[:, c0 - ic0:c0 - ic0 + nj * 8],
                              num_idxs=nj * 128,
                              num_idxs_reg=nj * 128,
                              elem_size=HW)
                      nc.vector.tensor_tensor(et4[:, :, :], et4[:, :, :],
                                              hsg4[:, :, 0:C], op=AL.add)

                      # r = relu(h_src + e); ex = exp(t*r + t*eps) in bf16;
                      # num-side = ex * r (the +eps on m is folded into h2)
                      nc.scalar.activation(et4[:, :, :], et4[:, :, :], ACTF.Relu)
                      exm4 = epool.tile([128, SK, C2], BF16, name="exm4",
                                        tag="exm",
                                        padded_shape=[128, SKMAX, C2])
                      nc.scalar.activation(exm4[:, :, C:C2], et4[:, :, :],
                                           ACTF.Exp, bias=teps_bc[:, l:l + 1],
                                           scale=t_bc[:, l:l + 1])
                      nc.scalar.activation(exm4[:, :, 0:C], et4[:, :, :],
                                           ACTF.Copy)
                      nc.vector.tensor_tensor(exm4[:, :, 0:C], exm4[:, :, 0:C],
                                              exm4[:, :, C:C2], op=AL.mult)

                      # --- per tile: one-hot scatter + h2 + transpose + MLP ---
                      h2qT = npool.tile([C, 512], F32, name="h2qT", tag="h2qT")
                      for t in range(gs):
                          frs = cfg.tfrag[Gi][t]
                          TK = sum(K for _, K in frs)
                          nmm = 0
                          eps_t = ps_edge.tile([128, C2], F32, name="eps_t",
                                               tag="eps_t")
                          for (kk0, K) in frs:
                              for j0 in range(0, K, 4):
                                  jj = min(4, K - j0)
                                  oh4 = epool.tile([128, 4, 128], BF16,
                                                   name="oh4", tag="oh4")
                                  nc.vector.tensor_tensor(
                                      oh4[:, :jj, :],
                                      dlt4[:, kk0 + j0:kk0 + j0 + jj]
                                      .to_broadcast([128, jj, 128]),
                                      iota4b[:, :jj, :], op=AL.is_equal)
                                  for q in range(jj):
                                      nc.tensor.matmul(
                                          eps_t[:, :], lhsT=oh4[:, q, :],
                                          rhs=exm4[:, kk0 + j0 + q, :],
                                          start=(nmm == 0),
                                          stop=(nmm == TK - 1))
                                      nmm += 1

                          # agg = num/max(den,1e-16) + eps;  h2 = h_own + agg
                          dinv = npool.tile([128, C], F32, name="dinv", tag="dinv")
                          nc.vector.tensor_scalar(dinv[:, :], eps_t[:, C:C2],
                                                  1e-16, None, AL.max)
                          nc.vector.reciprocal(dinv[:, :], dinv[:, :])
                          h2 = npool.tile([128, C], F32, name="h2", tag="h2")
                          nc.vector.tensor_tensor(h2[:, :], eps_t[:, 0:C],
                                                  dinv[:, :], op=AL.mult)
                          nc.vector.tensor_scalar(h2[:, :], h2[:, :], GEN_EPS,
                                                  None, AL.add)
                          nc.vector.tensor_tensor(h2[:, :], h2[:, :],
                                                  hog[:, t, :], op=AL.add)
                          emit_q(nc, h2[:, :])
                          trp = ps_tr.tile([C, 128], F32, name="trp", tag="tr")
                          nc.tensor.transpose(trp[:, :], h2[:, :],
                                              identity=ident[:, :])
                          nc.vector.tensor_copy(h2qT[:, t * 128:(t + 1) * 128],
                                                trp[:, :])

                      # --- MLP on the group (up to 4 node tiles) ---
                      w = gs * 128
                      z1p = ps_z1.tile([C2, 512], F32, name="z1p", tag="z1p")
                      nc.tensor.matmul(z1p[:, :w], lhsT=W1q[l][:, :],
                                       rhs=h2qT[:, :w], start=True, stop=True)
                      z1s = npool.tile([C2, 512], F32, name="z1s", tag="z1s")
                      nc.scalar.activation(z1s[:, :w], z1p[:, :w], ACTF.Identity,
                                           bias=bias1[l][:, :], scale=QS)
                      nc.scalar.activation(z1s[:, :w], z1s[:, :w], ACTF.Copy,
                                           bias=QB2, scale=QI)
                      nc.vector.tensor_scalar(z1s[:, :w], z1s[:, :w], QMAX, QMIN,
                                              AL.min, AL.max)
                      nc.scalar.activation(z1s[:, :w], z1s[:, :w], ACTF.Relu,
                                           bias=bi1[l][:, :], scale=sc1[l][:, :])
                      nc.scalar.activation(z1s[:, :w], z1s[:, :w], ACTF.Copy,
                                           bias=MAGIC, scale=QS)
                      nc.scalar.activation(z1s[:, :w], z1s[:, :w], ACTF.Copy,
                                           bias=QB2, scale=QI)
                      nc.vector.tensor_scalar(z1s[:, :w], z1s[:, :w], QMAX, QMIN,
                                              AL.min, AL.max)
                      z2p = ps_z2.tile([C, 512], F32, name="z2p", tag="z2p")
                      nc.tensor.matmul(z2p[:, :w], lhsT=W2q[l][:, :],
                                       rhs=z1s[:, :w], start=True, stop=True)
                      z2s = npool.tile([C, 512], F32, name="z2s", tag="z2s")
                      nc.scalar.activation(z2s[:, :w], z2p[:, :w], ACTF.Identity,
                                           bias=bias2[l][:, :], scale=QS)
                      nc.scalar.activation(z2s[:, :w], z2s[:, :w], ACTF.Copy,
                                           bias=QB2, scale=QI)
                      nc.vector.tensor_scalar(z2s[:, :w], z2s[:, :w], QMAX, QMIN,
                                              AL.min, AL.max)
                      hnext = npool.tile([128, GRP, C], F32, name="hnext",
                                         tag="hnext")
                      for q in range(gs):
                          trq = ps_tr.tile([128, C], F32, name="trq", tag="tr")
                          nc.tensor.transpose(trq[:, :],
                                              z2s[:, q * 128:(q + 1) * 128],
                                              identity=ident[0:C, 0:C])
                          nc.vector.tensor_tensor(hnext[:, q, :], trq[:, :],
                                                  hog[:, q, :], op=AL.add)
                          if last:
                              blt = npool.tile([128, 1], F32, name="blt", tag="blt")
                              nc.sync.dma_start(
                                  blt[:, :],
                                  d_bloc[(g0 + q) * 128:(g0 + q + 1) * 128]
                                  .rearrange("(p one) -> p one", one=1))
                              ohp = npool.tile([128, NW, 128], F32, name="ohp",
                                               tag="ohp")
                              nc.vector.tensor_tensor(
                                  ohp[:, :, :],
                                  blt[:, :].to_broadcast([128, NW, 128]),
                                  iota5[:, :, :], op=AL.is_equal)
                              for wi in range(NW):
                                  pps = ps_pool.tile([128, C], F32, name="pps",
                                                     tag="pps")
                                  nc.tensor.matmul(
                                      pps[:, :], lhsT=ohp[:, wi, :],
                                      rhs=hnext[:, q, :],
                                      start=True, stop=True)
                                  nc.vector.tensor_tensor(
                                      pacc[:, wi, :], pacc[:, wi, :], pps[:, :],
                                      op=AL.add)
                      if not last:
                          nc.sync.dma_start(
                              h_out[g0 * 128:(g0 + gs) * 128, 0:C]
                              .rearrange("(t p) c -> p t c", p=128),
                              hnext[:, :gs, :])

                  if not last:
                      if cfg.use_collectives:
                          nc.gpsimd.collective_compute(
                              "AllGather", AL.bypass, replica_groups=RG,
                              ins=[h_out[:, :]], outs=[h_fulls[l + 1][:, :]])
                      else:
                          for b_ in range(cfg.ncores):
                              nc.sync.dma_start(
                                  h_fulls[l + 1][b_ * NB:(b_ + 1) * NB, :],
                                  h_out[:, :])

              if cfg.n_layers < L:
                  nl = cfg.n_layers
                  hf = h_fulls[min(nl, L - 1)]
                  for b_ in range(NPAD // 128):
                      dbg_t = npool.tile([128, C], F32, name="dbg_t", tag="dbg_t")
                      nc.sync.dma_start(dbg_t[:, :],
                                        hf[b_ * 128:(b_ + 1) * 128, 0:C])
                      nc.sync.dma_start(d_hdbg[b_ * 128:(b_ + 1) * 128, :],
                                        dbg_t[:, :])
                  return nc

              # ---- pooling: write window partials, AllReduce, output head ----
              nc.sync.dma_start(
                  pool_glob[:, :].rearrange("(w p) c -> p w c", p=128),
                  pacc[:, :, :])
              if cfg.use_collectives:
                  nc.gpsimd.collective_compute(
                      "AllReduce", AL.add, replica_groups=RG,
                      ins=[pool_glob[:, :]], outs=[pool_red[:, :]])
              else:
                  nc.sync.dma_start(pool_red[:, :], pool_glob[:, :])

              n_out_tiles = (G + 127) // 128
              for i in range(n_out_tiles):
                  w = min(128, G - i * 128)
                  pt = npool.tile([128, C], F32, name="pt", tag="pt")
                  nc.sync.dma_start(pt[:w, :], pool_red[i * 128:i * 128 + w, :])
                  civ = npool.tile([128, 1], F32, name="civ", tag="civ")
                  nc.sync.dma_start(civ[:w, :],
                                    d_cntinv[i * 128:i * 128 + w].rearrange("(p one) -> p one", one=1))
                  nc.vector.tensor_scalar(pt[:w, :], pt[:w, :], civ[:w, :], None, AL.mult)
                  emit_q(nc, pt[:w, :])
                  trh = ps_tr.tile([C, 128], F32, name="trh", tag="tr")
                  nc.tensor.transpose(trh[:, :w], pt[:w, :], identity=ident[:w, :w])
                  hts = npool.tile([C, 128], F32, name="hts", tag="hts")
                  nc.vector.tensor_copy(hts[:, :w], trh[:, :w])
                  op_ = ps_z2.tile([1, 128], F32, name="op_", tag="z2p")
                  nc.tensor.matmul(op_[:, :w], lhsT=Woq[:, :], rhs=hts[:, :w],
                                   start=True, stop=True)
                  osb = npool.tile([1, 128], F32, name="osb", tag="osb")
                  nc.scalar.activation(osb[:, :w], op_[:, :w], ACTF.Identity,
                                       bias=biaso[:, :], scale=QS)
                  nc.scalar.activation(osb[:, :w], osb[:, :w], ACTF.Copy,
                                       bias=QB2, scale=QI)
                  nc.vector.tensor_scalar(osb[:, :w], osb[:, :w], QMAX, QMIN,
                                          AL.min, AL.max)
                  nc.scalar.activation(osb[:, :w], osb[:, :w], ACTF.Sigmoid)
                  nc.scalar.activation(osb[:, :w], osb[:, :w], ACTF.Copy,
                                       bias=MAGIC, scale=QS)
                  nc.scalar.activation(osb[:, :w], osb[:, :w], ACTF.Copy,
                                       bias=QB2, scale=QI)
                  nc.sync.dma_start(
                      d_out[i * 128:i * 128 + w, :].rearrange("w one -> one w"),
                      osb[:, :w])

    return nc


# ----------------------------------------------------------------------------
# Entry point.
# ----------------------------------------------------------------------------

def run(inputs, cfg, **run_kwargs):
    global LAST_RESULTS
    in_maps = preprocess(inputs, cfg)
    nc = build(cfg)
    if not nc.is_finalized():
        nc.finalize()
    res = run_bass_kernel_spmd(nc, in_maps, core_ids=list(range(cfg.ncores)),
                               **run_kwargs)
    LAST_RESULTS = res
    return res.results[0]["out"].reshape(cfg.G, 1).astype(np.float32)


def kernel(**inputs) -> np.ndarray:
    cfg = Cfg(N=100000, E=3200000, G=512, XD=8, ED=4, C=32, L=4)
    return run(inputs, cfg)
